# revision 63
# baseline (speedup 1.0000x reference)
"""Trainium2 Bass kernel for a BasicTransformerBlock (self-attn + cross-attn + GEGLU FF).

Sharding: data-parallel over the batch axis — 8 batch elements onto 8 NeuronCores,
same SPMD program, no collectives.

Design (driven by the TRN2 timeline cost model):
- Residual stream stays TRANSPOSED in SBUF as xT[d, s] (features on partitions,
  f32r), so every linear runs straight off the HBM weight layout; only the
  kernel entry/exit transpose via the PE (f32r identity).
- Heavy matmuls run in fp8e4 with DoubleRow perf mode: one instruction
  contracts TWO 128-row k-tiles at 0.5 cycles per moving element (4x fp32r).
  Weights are scaled by 32 on the f32->fp8 convert (sigma-0.02 weights would
  hit fp8 subnormals); the 1/32 descale rides the PSUM->SBUF copies that must
  exist anyway. LN gains g are folded into the same converts for free, so the
  LayerNorm itself only subtracts mu, multiplies rstd and adds b — three fused
  [128, 4, 512] DVE passes using zero-stride broadcast APs.
- Attention: scores in plain fp8 (K=64/head); exp reads score-PSUM pairs
  [128, 2, 512] in one ACT instruction and writes probs directly in fp8;
  probs@v uses DoubleRow over paired source-token tiles with the softmax
  denominator riding as a ones column of v (pv row 64); one reciprocal per
  (chunk, head-pair) on a par-indexed two-bank pv tile.
- Engines have in-order queues, so emission order is the schedule: wg/wf
  weight DMA+convert pairs are interleaved into the attention loops (attn1 is
  exp/ACT-bound for ~60us, so Pool does the converts there); kT2/v2 are
  computed during attn1; out-projections and the NEXT LayerNorm are emitted
  per-chunk from inside the attention loop (chunk_cb) to pipeline phases.
"""
import sys

sys.path.insert(0, "/opt/trn_rl_repo")

from contextlib import ExitStack

import numpy as np

import concourse.bass as bass
import concourse.mybir as mybir
import concourse.tile as tile
from concourse import bacc
from concourse.bass_utils import run_bass_kernel_spmd
from concourse.masks import make_identity

F32 = mybir.dt.float32
F32R = mybir.dt.float32r
BF16 = mybir.dt.bfloat16
F8 = mybir.dt.float8e4
AF = mybir.ActivationFunctionType
ALU = mybir.AluOpType
DR = mybir.MatmulPerfMode.DoubleRow

B = 8
S = 1024          # tokens
D = 512           # model dim
SK2 = 77          # cross-attention source length
DE = 768          # encoder dim
FF = 2048         # GEGLU inner dim (per half)
NH = 8            # heads
DH = 64           # head dim
SCALE = DH ** -0.5
EPS = 1e-5
P = 128
NC_ = 512         # token chunk (one psum bank of fp32)
ST = S // P       # 8 token tiles
FT = D // P       # 4 feature tiles
CH = S // NC_     # 2 token chunks
KE = DE // P      # 6 encoder feature tiles
NI = FF // P      # 16 FF inner tiles
WS = 32.0         # fp8 weight scale (keeps sigma~0.02 weights out of subnormals)
WSI = 1.0 / WS
EPAD = 128        # padded encT row pitch


def build(nc: bass.Bass):
    x = nc.dram_tensor("x", [S, D], F32, kind="ExternalInput")
    enc = nc.dram_tensor("enc", [SK2, DE], F32, kind="ExternalInput")
    w_in = {}
    for name, shape in [
        ("wq1", [D, D]), ("wk1", [D, D]), ("wv1", [D, D]), ("wo1", [D, D]),
        ("wq2", [D, D]), ("wk2", [DE, D]), ("wv2", [DE, D]), ("wo2", [D, D]),
        ("wg", [D, 2 * FF]), ("wf", [FF, D]),
    ]:
        w_in[name] = nc.dram_tensor(name, shape, F32, kind="ExternalInput")
    vec_in = {}
    for name, n in [("ln1_g", D), ("ln1_b", D), ("ln2_g", D), ("ln2_b", D),
                    ("ln3_g", D), ("ln3_b", D), ("bo1", D), ("bo2", D),
                    ("bg", 2 * FF), ("bf", D)]:
        vec_in[name] = nc.dram_tensor(name, [n], F32, kind="ExternalInput")
    out = nc.dram_tensor("out", [S, D], F32, kind="ExternalOutput")

    with tile.TileContext(nc) as tc, ExitStack() as es:
        const = es.enter_context(tc.tile_pool(name="const", bufs=1))
        resid = es.enter_context(tc.tile_pool(name="resid", bufs=2))
        stage = es.enter_context(tc.tile_pool(name="stage", bufs=2))
        wgstg = es.enter_context(tc.tile_pool(name="wgstg", bufs=4))
        wfstg = es.enter_context(tc.tile_pool(name="wfstg", bufs=1))
        rowp = es.enter_context(tc.tile_pool(name="rowp", bufs=1))
        bcp = es.enter_context(tc.tile_pool(name="bcp", bufs=2))
        tmp = es.enter_context(tc.tile_pool(name="tmp", bufs=2))
        lnt = es.enter_context(tc.tile_pool(name="lnt", bufs=1))
        actp = es.enter_context(tc.tile_pool(name="actp", bufs=1))

        # ---- constants ----
        ident_f = const.tile([P, P], F32)
        make_identity(nc, ident_f[:])
        ones_f = const.tile([P, P], F32)
        nc.vector.memset(ones_f[:], 1.0)
        zeros_f = const.tile([P, P], F32)
        nc.vector.memset(zeros_f[:], 0.0)
        ones128 = const.tile([P, 1], F32R)         # stats lhsT (K=128, M=1)
        nc.vector.tensor_copy(ones128[:], ones_f[:, 0:1])
        eps_t = const.tile([1, 1], F32)
        nc.vector.memset(eps_t[:], EPS)
        ws_t = const.tile([P, NC_], F32)           # x32 tile for Pool converts
        nc.vector.memset(ws_t[:], WS)
        ident_r = const.tile([P, P], F32R)
        nc.vector.tensor_copy(ident_r[:], ident_f[:])

        def col_const(name, n):
            t = const.tile([P, n], F32, tag=f"{name}_c")
            nc.sync.dma_start(t[:], vec_in[name].rearrange("(o p) -> p o", p=P))
            return t

        g1c, b1c = col_const("ln1_g", FT), col_const("ln1_b", FT)
        g2c, b2c = col_const("ln2_g", FT), col_const("ln2_b", FT)
        g3c, b3c = col_const("ln3_g", FT), col_const("ln3_b", FT)
        bo1c, bo2c = col_const("bo1", FT), col_const("bo2", FT)
        bfc = col_const("bf", FT)
        bgc = col_const("bg", 2 * NI)   # [:, 0:16]=u biases, [:, 16:32]=g biases
        # u-side bias prescaled by WS (descale happens at the wf epilogue)
        bguc = const.tile([P, NI], F32, tag="bgu32")
        nc.vector.tensor_scalar_mul(bguc[:], bgc[:, 0:NI], WS)

        # ---- load x, PE-transpose into xT [128, FT, S] (f32r) ----
        xT = resid.tile([P, FT, S], F32R, tag="x")
        encT = const.tile([P, KE, EPAD], F8, tag="encT")
        with tc.tile_pool(name="ps_in", bufs=2, space="PSUM") as ps_in, \
             tc.tile_pool(name="encstg", bufs=1) as encstg:
            for st in range(ST):
                xr = stage.tile([P, D], F32, tag="x_raw")
                nc.sync.dma_start(xr[:], x[P * st:P * (st + 1), :])
                pt = ps_in.tile([P, NC_], F32, tag="t")
                for ft in range(FT):
                    nc.tensor.transpose(pt[:, P * ft:P * (ft + 1)],
                                        xr[:, P * ft:P * (ft + 1)], ident_f[:])
                nc.vector.tensor_copy(
                    xT[:, :, P * st:P * (st + 1)],
                    pt[:].rearrange("p (f q) -> p f q", f=FT))

            # ---- enc: PE transposes into encT fp8 (padded pitch) ----
            enc_raw = encstg.tile([SK2, DE], F32, tag="enc_raw")
            nc.sync.dma_start(enc_raw[:], enc[:, :])
            for ke in range(KE):
                pt = ps_in.tile([P, NC_], F32, tag="t")
                nc.tensor.transpose(pt[:, 0:SK2],
                                    enc_raw[:, P * ke:P * (ke + 1)],
                                    ident_f[0:SK2, 0:SK2])
                nc.vector.tensor_copy(encT[:, ke, SK2:EPAD],
                                      zeros_f[:, 0:EPAD - SK2])
                nc.vector.tensor_copy(encT[:, ke, 0:SK2], pt[:, 0:SK2])

        def load_w8(pool, name, kouter, tag, eng, gcol=None):
            """Stream a [K, N<=512] HBM weight into [128, kouter, N] fp8 (xWS),
            optionally folding a per-input-feature LN gain g into the rows."""
            dram = w_in[name]
            nout = dram.shape[1]
            wr = pool.tile([P, kouter, nout], F8, tag=tag)
            dram_r = dram.rearrange("(ko ki) n -> ki ko n", ki=P)
            half = (kouter + 1) // 2 if kouter > 4 else kouter
            for k0 in range(0, kouter, half):
                k1 = min(k0 + half, kouter)
                stg = stage.tile([P, half, nout], F32, tag="wst")
                nc.sync.dma_start(stg[:, 0:k1 - k0, :], dram_r[:, k0:k1, :])
                if gcol is not None:
                    for ko in range(k0, k1):
                        nc.vector.tensor_scalar(
                            wr[:, ko, :], stg[:, ko - k0, :],
                            gcol[:, ko:ko + 1], WS, op0=ALU.mult, op1=ALU.mult)
                elif eng is nc.scalar:
                    eng.mul(wr[:, k0:k1, :], stg[:, 0:k1 - k0, :], WS)
                else:
                    eng.tensor_scalar_mul(wr[:, k0:k1, :], stg[:, 0:k1 - k0, :], WS)
            return wr

        def ln_chunk(src, bcol, hT, lps, c, st_tag="st"):
            """Emit LayerNorm chunk c: src f32r -> hT fp8 (g folded into the
            consuming weights, +b applied here). Stats share one psum bank."""
            cs = slice(NC_ * c, NC_ * (c + 1))
            st_ps = lps.tile([1, NC_], F32, tag=st_tag)
            for ft in range(FT):
                nc.tensor.matmul(st_ps[:], ones128[:], src[:, ft, cs],
                                 start=(ft == 0), stop=(ft == FT - 1))
            mu = rowp.tile([1, NC_], F32, tag="mu")
            nc.vector.tensor_scalar_mul(mu[:], st_ps[:], 1.0 / D)
            sq_ps = lps.tile([1, NC_], F32, tag=st_tag, name="sqps")
            for ft in range(FT):
                xsq = tmp.tile([P, NC_], F32R, tag="xsq")
                nc.scalar.activation(xsq[:], src[:, ft, cs], AF.Square)
                nc.tensor.matmul(sq_ps[:], ones128[:], xsq[:],
                                 start=(ft == 0), stop=(ft == FT - 1))
            musq = rowp.tile([1, NC_], F32, tag="musq")
            nc.vector.tensor_mul(musq[:], mu[:], mu[:])
            var = rowp.tile([1, NC_], F32, tag="var")
            nc.vector.scalar_tensor_tensor(
                var[:], sq_ps[:], 1.0 / D, musq[:],
                op0=ALU.mult, op1=ALU.subtract)
            sd = rowp.tile([1, NC_], F32, tag="sd")
            nc.scalar.activation(sd[:], var[:], AF.Sqrt, bias=eps_t[:])
            rstd = rowp.tile([1, NC_], F32, tag="rstd")
            nc.vector.reciprocal(rstd[:], sd[:])
            mu_b = bcp.tile([P, NC_], F32, tag="mub")
            nc.gpsimd.partition_broadcast(mu_b[:], mu[:])
            rstd_b = bcp.tile([P, NC_], F32, tag="rstdb")
            nc.gpsimd.partition_broadcast(rstd_b[:], rstd[:])
            t = lnt.tile([P, FT, NC_], F32R, tag="lt")
            nc.vector.tensor_tensor(
                t[:], src[:, :, cs],
                mu_b[:, None, :].broadcast_to([P, FT, NC_]), ALU.subtract)
            nc.vector.tensor_tensor(
                t[:], t[:],
                rstd_b[:, None, :].broadcast_to([P, FT, NC_]), ALU.mult)
            nc.vector.tensor_tensor(
                hT[:, :, cs], t[:],
                bcol[:, :, None].broadcast_to([P, FT, NC_]), ALU.add)

        def layer_norm(src, bcol, pool, tag):
            hT = pool.tile([P, FT, S], F8, tag=tag, name=f"h_{tag}")
            with tc.tile_pool(name=f"ps_{tag}", bufs=2, space="PSUM") as lps:
                for c in range(CH):
                    ln_chunk(src, bcol, hT, lps, c)
            return hT

        def project_dr(w_r, src, pool, tag, copy_eng="dve"):
            """yT = (W*WS).T @ src via fp8 DoubleRow, descaled 1/WS on copy.
            src [128, FT, S] fp8; w_r [128, FT, D] fp8. -> [128, FT, S] fp8."""
            yT = pool.tile([P, FT, S], F8, tag=tag)
            with tc.tile_pool(name=f"ps_{tag}", bufs=2, space="PSUM") as pps:
                for c in range(CH):
                    cs = slice(NC_ * c, NC_ * (c + 1))
                    for mo in range(FT):
                        pt = pps.tile([P, NC_], F32, tag="t")
                        for j in range(FT // 2):
                            nc.tensor.matmul(
                                pt[:], w_r[:, 2 * j:2 * j + 2, P * mo:P * (mo + 1)],
                                src[:, 2 * j:2 * j + 2, cs],
                                start=(j == 0), stop=(j == FT // 2 - 1),
                                perf_mode=DR)
                        if copy_eng == "act":
                            nc.scalar.mul(yT[:, mo, cs], pt[:], WSI)
                        else:
                            nc.vector.tensor_scalar_mul(yT[:, mo, cs], pt[:], WSI)
            return yT

        def attention_into(qT, kT, v_sb, n_sk, aT, aT_tag, hook=None,
                           chunk_cb=None):
            """qT/kT: [128, FT, S*] fp8 transposed; v_sb: [part, sk_tiles, NH,
            DH+2] fp8 with ones in col DH. Fills aT [128, FT, S] fp8.
            hook(slot) emits interleaved filler work; chunk_cb(c) emits the
            per-chunk consumer (out-projection) as soon as chunk c is done."""
            sk_tiles = (n_sk + P - 1) // P
            with tc.tile_pool(name=f"ps_sc_{aT_tag}", bufs=2, space="PSUM") as psc, \
                 tc.tile_pool(name=f"ps_pv_{aT_tag}", bufs=1, space="PSUM") as ppv, \
                 tc.tile_pool(name=f"ex_{aT_tag}", bufs=2) as exp_pool:
                for c in range(CH):
                    for hf in range(FT):
                        cs = slice(NC_ * c, NC_ * (c + 1))
                        ex3 = exp_pool.tile([P, sk_tiles, 2, NC_], F8, tag="ex")
                        for sk in range(sk_tiles):
                            rows = min(P, n_sk - P * sk)
                            sp = psc.tile([P, 2, NC_], F32, tag="sp")
                            for par in range(2):
                                hp = slice(DH * par, DH * par + DH)
                                nc.tensor.matmul(
                                    sp[:rows, par, :],
                                    kT[hp, hf, P * sk:P * sk + rows],
                                    qT[hp, hf, cs], start=True, stop=True)
                            nc.scalar.activation(ex3[:rows, sk, :, :],
                                                 sp[:rows, :, :], AF.Exp,
                                                 scale=SCALE)
                        pv = ppv.tile([DH + 2, 2, NC_], F32, tag="pv")
                        for par in range(2):
                            h = 2 * hf + par
                            if sk_tiles % 2 == 0:
                                for j in range(sk_tiles // 2):
                                    nc.tensor.matmul(
                                        pv[:, par, :],
                                        v_sb[:, 2 * j:2 * j + 2, h, :],
                                        ex3[:, 2 * j:2 * j + 2, par, :],
                                        start=(j == 0),
                                        stop=(j == sk_tiles // 2 - 1),
                                        perf_mode=DR)
                            else:
                                for sk in range(sk_tiles):
                                    rows = min(P, n_sk - P * sk)
                                    nc.tensor.matmul(
                                        pv[:, par, :], v_sb[:rows, sk, h, :],
                                        ex3[:rows, sk, par, :],
                                        start=(sk == 0), stop=(sk == sk_tiles - 1))
                        rc = rowp.tile([1, 2, NC_], F32, tag="rc")
                        nc.vector.reciprocal(rc[:], pv[DH:DH + 1, :, :])
                        for par in range(2):
                            hp = slice(DH * par, DH * par + DH)
                            den_b = bcp.tile([DH, NC_], F32, tag="denb")
                            nc.gpsimd.partition_broadcast(den_b[:], rc[0:1, par, :])
                            nc.vector.tensor_tensor(aT[hp, hf, cs],
                                                    pv[0:DH, par, :],
                                                    den_b[:], ALU.mult)
                        if hook is not None:
                            hook(c * FT + hf)
                    if chunk_cb is not None:
                        chunk_cb(c)

        def out_proj_chunk(ops, w_r, bias_c, aT, src, dst, c):
            """dst[:, :, chunk c] = src + (W*WS).T @ aT / WS + bias."""
            cs = slice(NC_ * c, NC_ * (c + 1))
            for mo in range(FT):
                pt = ops.tile([P, NC_], F32, tag="t")
                for j in range(FT // 2):
                    nc.tensor.matmul(
                        pt[:], w_r[:, 2 * j:2 * j + 2, P * mo:P * (mo + 1)],
                        aT[:, 2 * j:2 * j + 2, cs],
                        start=(j == 0), stop=(j == FT // 2 - 1),
                        perf_mode=DR)
                t1 = tmp.tile([P, NC_], F32R, tag="epi")
                nc.vector.tensor_scalar(
                    t1[:], pt[:], WSI, bias_c[:, mo:mo + 1],
                    op0=ALU.mult, op1=ALU.add)
                nc.gpsimd.tensor_add(dst[:, mo, cs], t1[:], src[:, mo, cs])

        # ================= self-attention =================
        wpool = es.enter_context(tc.tile_pool(name="wpool", bufs=1))
        wq1r = load_w8(wpool, "wq1", FT, "wq1r", nc.vector, gcol=g1c)
        wk1r = load_w8(wpool, "wk1", FT, "wk1r", nc.vector, gcol=g1c)
        wv1r = load_w8(wpool, "wv1", FT, "wv1r", nc.vector, gcol=g1c)
        with tc.tile_pool(name="a1", bufs=1) as a1:
            h1 = layer_norm(xT, b1c, a1, "h1")
            # attn2 enc-side weights: convert on ACT before the exp phase starts
            wk2r = load_w8(wpool, "wk2", KE, "wk2r", nc.scalar)
            wv2r = load_w8(wpool, "wv2", KE, "wv2r", nc.scalar)
            qT1 = project_dr(wq1r, h1, a1, "qT1")
            kT1 = project_dr(wk1r, h1, a1, "kT1")
            v1 = a1.tile([P, ST, NH, DH + 2], F8, tag="v1")
            nc.vector.tensor_copy(
                v1[:, :, :, DH:DH + 1],
                ones_f[:, 0:ST * NH].rearrange("p (a b c) -> p a b c",
                                               a=ST, b=NH))
            nc.vector.tensor_copy(
                v1[:, :, :, DH + 1:DH + 2],
                zeros_f[:, 0:ST * NH].rearrange("p (a b c) -> p a b c",
                                                a=ST, b=NH))
            with tc.tile_pool(name="ps_v1", bufs=2, space="PSUM") as vps:
                for st in range(ST):
                    pt = vps.tile([P, NC_], F32, tag="t")
                    for j in range(FT // 2):
                        nc.tensor.matmul(
                            pt[:], h1[:, 2 * j:2 * j + 2, P * st:P * (st + 1)],
                            wv1r[:, 2 * j:2 * j + 2, :],
                            start=(j == 0), stop=(j == FT // 2 - 1),
                            perf_mode=DR)
                    nc.vector.tensor_scalar_mul(
                        v1[:, st, :, 0:DH],
                        pt[:].rearrange("p (h d) -> p h d", h=NH), WSI)

            wo1r = load_w8(wpool, "wo1", FT, "wo1r", nc.vector)
            wq2r = load_w8(wpool, "wq2", FT, "wq2r", nc.vector, gcol=g2c)
            wo2r = load_w8(wpool, "wo2", FT, "wo2r", nc.vector)

            # kT2/v2 depend only on enc: compute during attn1
            kT2 = wpool.tile([P, FT, EPAD], F8, tag="kT2")
            v2 = wpool.tile([SK2, 1, NH, DH + 2], F8, tag="v2")
            nc.vector.tensor_copy(
                v2[:, :, :, DH:DH + 1],
                ones_f[0:SK2, 0:NH].rearrange("p (a b c) -> p a b c", a=1, b=NH))
            nc.vector.tensor_copy(
                v2[:, :, :, DH + 1:DH + 2],
                zeros_f[0:SK2, 0:NH].rearrange("p (a b c) -> p a b c", a=1, b=NH))
            with tc.tile_pool(name="ps_kv2", bufs=2, space="PSUM") as kvps:
                for mo in range(FT):
                    pt = kvps.tile([P, NC_], F32, tag="t")
                    for j in range(KE // 2):
                        nc.tensor.matmul(
                            pt[:, 0:EPAD],
                            wk2r[:, 2 * j:2 * j + 2, P * mo:P * (mo + 1)],
                            encT[:, 2 * j:2 * j + 2, :],
                            start=(j == 0), stop=(j == KE // 2 - 1),
                            perf_mode=DR)
                    nc.scalar.mul(kT2[:, mo, :], pt[:, 0:EPAD], WSI)
                pt = kvps.tile([P, NC_], F32, tag="t")
                for j in range(KE // 2):
                    nc.tensor.matmul(pt[:, :], encT[:, 2 * j:2 * j + 2, :],
                                     wv2r[:, 2 * j:2 * j + 2, :],
                                     start=(j == 0), stop=(j == KE // 2 - 1),
                                     perf_mode=DR)
                nc.scalar.mul(
                    v2[:, 0, :, 0:DH],
                    pt[0:SK2, :].rearrange("p (h d) -> p h d", h=NH), WSI)

            # wg streamed + converted on Pool while attn1's ACT runs exp
            wgur = wpool.tile([P, NI, FT, P], F8, tag="wgur")
            wggr = wpool.tile([P, NI, FT, P], F8, tag="wggr")
            wg_r = w_in["wg"].rearrange("(ko ki) n -> ki ko n", ki=P)
            ws_g3 = const.tile([P, FT, P], F32, tag="ws_g3")
            for ft in range(FT):
                nc.vector.tensor_scalar(
                    ws_g3[:, ft, :], ones_f[:, 0:P], g3c[:, ft:ft + 1], WS,
                    op0=ALU.mult, op1=ALU.mult)
            ws_v = ws_g3[:]

            def wg_hook(slot):
                for i in range(2 * slot, 2 * slot + 2):
                    wgu = wgstg.tile([P, FT, P], F32, tag="wgst")
                    nc.sync.dma_start(wgu[:], wg_r[:, :, P * i:P * (i + 1)])
                    nc.gpsimd.tensor_tensor(wgur[:, i, :, :], wgu[:], ws_v,
                                            ALU.mult)
                    wgg = wgstg.tile([P, FT, P], F32, tag="wgst")
                    nc.sync.dma_start(
                        wgg[:], wg_r[:, :, FF + P * i:FF + P * (i + 1)])
                    nc.gpsimd.tensor_tensor(wggr[:, i, :, :], wgg[:], ws_v,
                                            ALU.mult)

            xT1 = resid.tile([P, FT, S], F32R, tag="x")
            h2 = actp.tile([P, FT, S], F8, tag="h2")
            aT1_box = []

            def o1_cb(c):
                out_proj_chunk(ops1, wo1r, bo1c, aT1_box[0], xT, xT1, c)
                ln_chunk(xT1, b2c, h2, ops1, c)

            with tc.tile_pool(name="ps_o1", bufs=1, space="PSUM") as ops1:
                aT1_box.append(a1.tile([P, FT, S], F8, tag="aT1", name="aT1"))
                attention_into(qT1, kT1, v1, S, aT1_box[0], "aT1", hook=wg_hook,
                               chunk_cb=o1_cb)

        # ================= cross-attention =================
        with tc.tile_pool(name="a2", bufs=1) as a2:
            qT2 = project_dr(wq2r, h2, a2, "qT2", copy_eng="act")

            # wf streamed + converted on Pool during attn2
            wfr = wpool.tile([P, NI, D], F8, tag="wfr")
            wf_r = w_in["wf"].rearrange("(ko ki) n -> ki ko n", ki=P)

            def wf_hook(slot):
                if slot >= FT:
                    return
                kq = slot
                wf_stg = wfstg.tile([P, FT, D], F32, tag="wfst")
                nc.sync.dma_start(wf_stg[:], wf_r[:, FT * kq:FT * (kq + 1), :])
                for ko in range(FT):
                    nc.gpsimd.tensor_tensor(
                        wfr[:, FT * kq + ko, :], wf_stg[:, ko, :], ws_t[:],
                        ALU.mult)

            xT2 = resid.tile([P, FT, S], F32R, tag="x")
            h3 = actp.tile([P, FT, S], F8, tag="h3")
            aT2_box = []

            def o2_cb(c):
                out_proj_chunk(ops2, wo2r, bo2c, aT2_box[0], xT1, xT2, c)
                ln_chunk(xT2, b3c, h3, ops2, c)

            with tc.tile_pool(name="ps_o2", bufs=1, space="PSUM") as ops2:
                aT2_box.append(a2.tile([P, FT, S], F8, tag="aT2", name="aT2"))
                attention_into(qT2, kT2, v2, SK2, aT2_box[0], "aT2", hook=wf_hook,
                               chunk_cb=o2_cb)

        # ================= GEGLU feed-forward =================
        with tc.tile_pool(name="ffp", bufs=1) as ffp, \
             tc.tile_pool(name="fft_p", bufs=4) as fft_p, \
             tc.tile_pool(name="gel_p", bufs=4) as gel_p:
            xT3 = resid.tile([P, FT, S], F32R, tag="x")
            with tc.tile_pool(name="ps_ff", bufs=4, space="PSUM") as ffps, \
                 tc.tile_pool(name="ps_wf", bufs=4, space="PSUM") as wfps:
                for c in range(CH):
                    cs = slice(NC_ * c, NC_ * (c + 1))
                    wf_ps = [wfps.tile([P, NC_], F32, tag="a",
                                       name=f"wf_ps{c}_{m}") for m in range(FT)]
                    for i in range(NI):
                        pu = ffps.tile([P, NC_], F32, tag="t")
                        for j in range(FT // 2):
                            nc.tensor.matmul(pu[:],
                                             wgur[:, i, 2 * j:2 * j + 2, :],
                                             h3[:, 2 * j:2 * j + 2, cs],
                                             start=(j == 0), stop=(j == FT // 2 - 1),
                                             perf_mode=DR)
                        pg = ffps.tile([P, NC_], F32, tag="t")
                        for j in range(FT // 2):
                            nc.tensor.matmul(pg[:],
                                             wggr[:, i, 2 * j:2 * j + 2, :],
                                             h3[:, 2 * j:2 * j + 2, cs],
                                             start=(j == 0), stop=(j == FT // 2 - 1),
                                             perf_mode=DR)
                        gel = gel_p.tile([P, NC_], BF16, tag="gel")
                        nc.scalar.activation(gel[:], pg[:], AF.Gelu,
                                             bias=bgc[:, NI + i:NI + i + 1],
                                             scale=WSI)
                        if i % 2 == 0:
                            fpair = fft_p.tile([P, 2, NC_], F8, tag="fpair")
                        # fft = (pu + 32*bg_u) * gel = 32 * (u+bu)*gelu(g+bgg)
                        nc.vector.scalar_tensor_tensor(
                            fpair[:, i % 2, :], pu[:], bguc[:, i:i + 1], gel[:],
                            op0=ALU.add, op1=ALU.mult)
                        if i % 2 == 1:
                            for m in range(FT):
                                nc.tensor.matmul(
                                    wf_ps[m][:],
                                    wfr[:, i - 1:i + 1, P * m:P * (m + 1)],
                                    fpair[:], start=(i == 1), stop=(i == NI - 1),
                                    perf_mode=DR, skip_group_check=True)
                    for m in range(FT):
                        t1 = tmp.tile([P, NC_], F32R, tag="epi")
                        nc.vector.tensor_scalar(
                            t1[:], wf_ps[m][:], WSI * WSI, bfc[:, m:m + 1],
                            op0=ALU.mult, op1=ALU.add)
                        nc.gpsimd.tensor_add(xT3[:, m, cs], t1[:], xT2[:, m, cs])
                    # transpose back & store this chunk immediately
                    for st in range(4 * c, 4 * c + 4):
                        ot = stage.tile([P, D], F32, tag="x_raw")
                        pt = ffps.tile([P, NC_], F32R, tag="t")
                        for ft in range(FT):
                            nc.tensor.transpose(pt[:, P * ft:P * (ft + 1)],
                                                xT3[:, ft, P * st:P * (st + 1)],
                                                ident_r[:])
                        nc.vector.tensor_copy(ot[:], pt[:])
                        nc.sync.dma_start(out[P * st:P * (st + 1), :], ot[:])

    return nc


_CACHED = {}


def _get_nc():
    if "nc" not in _CACHED:
        nc = bacc.Bacc("TRN2", target_bir_lowering=False, debug=False, num_devices=B)
        build(nc)
        nc.finalize()
        _CACHED["nc"] = nc
    return _CACHED["nc"]


def kernel(**inputs) -> np.ndarray:
    nc = _get_nc()
    x = np.ascontiguousarray(np.asarray(inputs["x"]), dtype=np.float32)
    enc = np.ascontiguousarray(np.asarray(inputs["enc"]), dtype=np.float32)
    shared = {k: np.ascontiguousarray(np.asarray(v), dtype=np.float32)
              for k, v in inputs.items() if k not in ("x", "enc")}
    in_maps = [dict(shared, x=x[i], enc=enc[i]) for i in range(B)]
    res = run_bass_kernel_spmd(nc, in_maps, core_ids=list(range(B)))
    outs = []
    for i in range(B):
        o = np.asarray(res.results[i]["out"])
        outs.append(o.view(np.float32) if o.dtype != np.float32 else o)
    return np.stack(outs, axis=0)


if __name__ == "__main__":
    print("module import ok")


# revision 64
# speedup vs baseline: 1.0110x; 1.0110x over previous
"""Trainium2 Bass kernel for a BasicTransformerBlock (self-attn + cross-attn + GEGLU FF).

Sharding: data-parallel over the batch axis — 8 batch elements onto 8 NeuronCores,
same SPMD program, no collectives.

Design (driven by the TRN2 timeline cost model):
- Residual stream stays TRANSPOSED in SBUF as xT[d, s] (features on partitions,
  f32r), so every linear runs straight off the HBM weight layout; only the
  kernel entry/exit transpose via the PE (f32r identity).
- Heavy matmuls run in fp8e4 with DoubleRow perf mode: one instruction
  contracts TWO 128-row k-tiles at 0.5 cycles per moving element (4x fp32r).
  Weights are scaled by 32 on the f32->fp8 convert (sigma-0.02 weights would
  hit fp8 subnormals); the 1/32 descale rides the PSUM->SBUF copies that must
  exist anyway. LN gains g are folded into the same converts for free, so the
  LayerNorm itself only subtracts mu, multiplies rstd and adds b — three fused
  [128, 4, 512] DVE passes using zero-stride broadcast APs.
- Attention: scores in plain fp8 (K=64/head); exp reads score-PSUM pairs
  [128, 2, 512] in one ACT instruction and writes probs directly in fp8;
  probs@v uses DoubleRow over paired source-token tiles with the softmax
  denominator riding as a ones column of v (pv row 64); one reciprocal per
  (chunk, head-pair) on a par-indexed two-bank pv tile.
- Engines have in-order queues, so emission order is the schedule: wg/wf
  weight DMA+convert pairs are interleaved into the attention loops (attn1 is
  exp/ACT-bound for ~60us, so Pool does the converts there); kT2/v2 are
  computed during attn1; out-projections and the NEXT LayerNorm are emitted
  per-chunk from inside the attention loop (chunk_cb) to pipeline phases.
"""
import sys

sys.path.insert(0, "/opt/trn_rl_repo")

from contextlib import ExitStack

import numpy as np

import concourse.bass as bass
import concourse.mybir as mybir
import concourse.tile as tile
from concourse import bacc
from concourse.bass_utils import run_bass_kernel_spmd
from concourse.masks import make_identity

F32 = mybir.dt.float32
F32R = mybir.dt.float32r
BF16 = mybir.dt.bfloat16
F8 = mybir.dt.float8e4
AF = mybir.ActivationFunctionType
ALU = mybir.AluOpType
DR = mybir.MatmulPerfMode.DoubleRow

B = 8
S = 1024          # tokens
D = 512           # model dim
SK2 = 77          # cross-attention source length
DE = 768          # encoder dim
FF = 2048         # GEGLU inner dim (per half)
NH = 8            # heads
DH = 64           # head dim
SCALE = DH ** -0.5
EPS = 1e-5
P = 128
NC_ = 512         # token chunk (one psum bank of fp32)
ST = S // P       # 8 token tiles
FT = D // P       # 4 feature tiles
CH = S // NC_     # 2 token chunks
KE = DE // P      # 6 encoder feature tiles
NI = FF // P      # 16 FF inner tiles
WS = 32.0         # fp8 weight scale (keeps sigma~0.02 weights out of subnormals)
WSI = 1.0 / WS
EPAD = 128        # padded encT row pitch


def build(nc: bass.Bass):
    x = nc.dram_tensor("x", [S, D], F32, kind="ExternalInput")
    enc = nc.dram_tensor("enc", [SK2, DE], F32, kind="ExternalInput")
    w_in = {}
    for name, shape in [
        ("wq1", [D, D]), ("wk1", [D, D]), ("wv1", [D, D]), ("wo1", [D, D]),
        ("wq2", [D, D]), ("wk2", [DE, D]), ("wv2", [DE, D]), ("wo2", [D, D]),
        ("wg", [D, 2 * FF]), ("wf", [FF, D]),
    ]:
        w_in[name] = nc.dram_tensor(name, shape, F32, kind="ExternalInput")
    vec_in = {}
    for name, n in [("ln1_g", D), ("ln1_b", D), ("ln2_g", D), ("ln2_b", D),
                    ("ln3_g", D), ("ln3_b", D), ("bo1", D), ("bo2", D),
                    ("bg", 2 * FF), ("bf", D)]:
        vec_in[name] = nc.dram_tensor(name, [n], F32, kind="ExternalInput")
    out = nc.dram_tensor("out", [S, D], F32, kind="ExternalOutput")

    with tile.TileContext(nc) as tc, ExitStack() as es:
        const = es.enter_context(tc.tile_pool(name="const", bufs=1))
        resid = es.enter_context(tc.tile_pool(name="resid", bufs=2))
        stage = es.enter_context(tc.tile_pool(name="stage", bufs=2))
        wgstg = es.enter_context(tc.tile_pool(name="wgstg", bufs=4))
        wfstg = es.enter_context(tc.tile_pool(name="wfstg", bufs=1))
        rowp = es.enter_context(tc.tile_pool(name="rowp", bufs=1))
        bcp = es.enter_context(tc.tile_pool(name="bcp", bufs=2))
        tmp = es.enter_context(tc.tile_pool(name="tmp", bufs=2))
        lnt = es.enter_context(tc.tile_pool(name="lnt", bufs=1))
        actp = es.enter_context(tc.tile_pool(name="actp", bufs=1))

        # ---- constants ----
        ident_f = const.tile([P, P], F32)
        make_identity(nc, ident_f[:])
        ones_f = const.tile([P, P], F32)
        nc.vector.memset(ones_f[:], 1.0)
        zeros_f = const.tile([P, P], F32)
        nc.vector.memset(zeros_f[:], 0.0)
        ones128 = const.tile([P, 1], F32R)         # stats lhsT (K=128, M=1)
        nc.vector.tensor_copy(ones128[:], ones_f[:, 0:1])
        eps_t = const.tile([1, 1], F32)
        nc.vector.memset(eps_t[:], EPS)
        ws_t = const.tile([P, NC_], F32)           # x32 tile for Pool converts
        nc.vector.memset(ws_t[:], WS)
        ident_r = const.tile([P, P], F32R)
        nc.vector.tensor_copy(ident_r[:], ident_f[:])

        def col_const(name, n):
            t = const.tile([P, n], F32, tag=f"{name}_c")
            nc.sync.dma_start(t[:], vec_in[name].rearrange("(o p) -> p o", p=P))
            return t

        g1c, b1c = col_const("ln1_g", FT), col_const("ln1_b", FT)
        g2c, b2c = col_const("ln2_g", FT), col_const("ln2_b", FT)
        g3c, b3c = col_const("ln3_g", FT), col_const("ln3_b", FT)
        bo1c, bo2c = col_const("bo1", FT), col_const("bo2", FT)
        bfc = col_const("bf", FT)
        bgc = col_const("bg", 2 * NI)   # [:, 0:16]=u biases, [:, 16:32]=g biases
        # u-side bias prescaled by WS (descale happens at the wf epilogue)
        bguc = const.tile([P, NI], F32, tag="bgu32")
        nc.vector.tensor_scalar_mul(bguc[:], bgc[:, 0:NI], WS)
        g1ws = const.tile([P, FT], F32, tag="g1ws")
        nc.vector.tensor_scalar_mul(g1ws[:], g1c[:], WS)
        g2ws = const.tile([P, FT], F32, tag="g2ws")
        nc.vector.tensor_scalar_mul(g2ws[:], g2c[:], WS)

        # ---- load x, PE-transpose into xT [128, FT, S] (f32r) ----
        xT = resid.tile([P, FT, S], F32R, tag="x")
        encT = const.tile([P, KE, EPAD], F8, tag="encT")
        with tc.tile_pool(name="ps_in", bufs=2, space="PSUM") as ps_in, \
             tc.tile_pool(name="encstg", bufs=1) as encstg:
            for st in range(ST):
                xr = stage.tile([P, D], F32, tag="x_raw")
                nc.sync.dma_start(xr[:], x[P * st:P * (st + 1), :])
                pt = ps_in.tile([P, NC_], F32, tag="t")
                for ft in range(FT):
                    nc.tensor.transpose(pt[:, P * ft:P * (ft + 1)],
                                        xr[:, P * ft:P * (ft + 1)], ident_f[:])
                nc.scalar.copy(
                    xT[:, :, P * st:P * (st + 1)],
                    pt[:].rearrange("p (f q) -> p f q", f=FT))

            # ---- enc: PE transposes into encT fp8 (padded pitch) ----
            enc_raw = encstg.tile([SK2, DE], F32, tag="enc_raw")
            nc.sync.dma_start(enc_raw[:], enc[:, :])
            for ke in range(KE):
                pt = ps_in.tile([P, NC_], F32, tag="t")
                nc.tensor.transpose(pt[:, 0:SK2],
                                    enc_raw[:, P * ke:P * (ke + 1)],
                                    ident_f[0:SK2, 0:SK2])
                nc.vector.tensor_copy(encT[:, ke, SK2:EPAD],
                                      zeros_f[:, 0:EPAD - SK2])
                nc.vector.tensor_copy(encT[:, ke, 0:SK2], pt[:, 0:SK2])

        def load_w8(pool, name, kouter, tag, eng, gcol=None, scale_col=None):
            """Stream a [K, N<=512] HBM weight into [128, kouter, N] fp8 (xWS),
            optionally folding a per-input-feature LN gain g into the rows
            (scale_col = g*WS columns, applied via the ACT scale pointer)."""
            dram = w_in[name]
            nout = dram.shape[1]
            wr = pool.tile([P, kouter, nout], F8, tag=tag)
            dram_r = dram.rearrange("(ko ki) n -> ki ko n", ki=P)
            half = (kouter + 1) // 2 if kouter > 4 else kouter
            for k0 in range(0, kouter, half):
                k1 = min(k0 + half, kouter)
                stg = stage.tile([P, half, nout], F32, tag="wst")
                nc.sync.dma_start(stg[:, 0:k1 - k0, :], dram_r[:, k0:k1, :])
                if scale_col is not None:
                    for ko in range(k0, k1):
                        nc.scalar.activation(
                            wr[:, ko, :], stg[:, ko - k0, :], AF.Copy,
                            scale=scale_col[:, ko:ko + 1])
                elif gcol is not None:
                    for ko in range(k0, k1):
                        nc.vector.tensor_scalar(
                            wr[:, ko, :], stg[:, ko - k0, :],
                            gcol[:, ko:ko + 1], WS, op0=ALU.mult, op1=ALU.mult)
                elif eng is nc.scalar:
                    eng.mul(wr[:, k0:k1, :], stg[:, 0:k1 - k0, :], WS)
                else:
                    eng.tensor_scalar_mul(wr[:, k0:k1, :], stg[:, 0:k1 - k0, :], WS)
            return wr

        def ln_chunk(src, bcol, hT, lps, c, st_tag="st"):
            """Emit LayerNorm chunk c: src f32r -> hT fp8 (g folded into the
            consuming weights, +b applied here). Stats share one psum bank."""
            cs = slice(NC_ * c, NC_ * (c + 1))
            st_ps = lps.tile([1, NC_], F32, tag=st_tag)
            for ft in range(FT):
                nc.tensor.matmul(st_ps[:], ones128[:], src[:, ft, cs],
                                 start=(ft == 0), stop=(ft == FT - 1))
            mu = rowp.tile([1, NC_], F32, tag="mu")
            nc.vector.tensor_scalar_mul(mu[:], st_ps[:], 1.0 / D)
            sq_ps = lps.tile([1, NC_], F32, tag=st_tag, name="sqps")
            for ft in range(FT):
                xsq = tmp.tile([P, NC_], F32R, tag="xsq")
                nc.scalar.activation(xsq[:], src[:, ft, cs], AF.Square)
                nc.tensor.matmul(sq_ps[:], ones128[:], xsq[:],
                                 start=(ft == 0), stop=(ft == FT - 1))
            musq = rowp.tile([1, NC_], F32, tag="musq")
            nc.vector.tensor_mul(musq[:], mu[:], mu[:])
            var = rowp.tile([1, NC_], F32, tag="var")
            nc.vector.scalar_tensor_tensor(
                var[:], sq_ps[:], 1.0 / D, musq[:],
                op0=ALU.mult, op1=ALU.subtract)
            sd = rowp.tile([1, NC_], F32, tag="sd")
            nc.scalar.activation(sd[:], var[:], AF.Sqrt, bias=eps_t[:])
            rstd = rowp.tile([1, NC_], F32, tag="rstd")
            nc.vector.reciprocal(rstd[:], sd[:])
            mu_b = bcp.tile([P, NC_], F32, tag="mub")
            nc.gpsimd.partition_broadcast(mu_b[:], mu[:])
            rstd_b = bcp.tile([P, NC_], F32, tag="rstdb")
            nc.gpsimd.partition_broadcast(rstd_b[:], rstd[:])
            t = lnt.tile([P, FT, NC_], F32R, tag="lt")
            nc.vector.tensor_tensor(
                t[:], src[:, :, cs],
                mu_b[:, None, :].broadcast_to([P, FT, NC_]), ALU.subtract)
            nc.vector.tensor_tensor(
                t[:], t[:],
                rstd_b[:, None, :].broadcast_to([P, FT, NC_]), ALU.mult)
            nc.vector.tensor_tensor(
                hT[:, :, cs], t[:],
                bcol[:, :, None].broadcast_to([P, FT, NC_]), ALU.add)

        def layer_norm(src, bcol, pool, tag):
            hT = pool.tile([P, FT, S], F8, tag=tag, name=f"h_{tag}")
            with tc.tile_pool(name=f"ps_{tag}", bufs=2, space="PSUM") as lps:
                for c in range(CH):
                    ln_chunk(src, bcol, hT, lps, c)
            return hT

        def project_dr(w_r, src, pool, tag, copy_eng="dve"):
            """yT = (W*WS).T @ src via fp8 DoubleRow, descaled 1/WS on copy.
            src [128, FT, S] fp8; w_r [128, FT, D] fp8. -> [128, FT, S] fp8."""
            yT = pool.tile([P, FT, S], F8, tag=tag)
            with tc.tile_pool(name=f"ps_{tag}", bufs=2, space="PSUM") as pps:
                for c in range(CH):
                    cs = slice(NC_ * c, NC_ * (c + 1))
                    for mo in range(FT):
                        pt = pps.tile([P, NC_], F32, tag="t")
                        for j in range(FT // 2):
                            nc.tensor.matmul(
                                pt[:], w_r[:, 2 * j:2 * j + 2, P * mo:P * (mo + 1)],
                                src[:, 2 * j:2 * j + 2, cs],
                                start=(j == 0), stop=(j == FT // 2 - 1),
                                perf_mode=DR)
                        if copy_eng == "act":
                            nc.scalar.mul(yT[:, mo, cs], pt[:], WSI)
                        else:
                            nc.vector.tensor_scalar_mul(yT[:, mo, cs], pt[:], WSI)
            return yT

        def attention_into(qT, kT, v_sb, n_sk, aT, aT_tag, hook=None,
                           chunk_cb=None, sc_bufs=2, pv_bufs=1):
            """qT/kT: [128, FT, S*] fp8 transposed; v_sb: [part, sk_tiles, NH,
            DH+2] fp8 with ones in col DH. Fills aT [128, FT, S] fp8.
            hook(slot) emits interleaved filler work; chunk_cb(c) emits the
            per-chunk consumer (out-projection) as soon as chunk c is done."""
            sk_tiles = (n_sk + P - 1) // P
            with tc.tile_pool(name=f"ps_sc_{aT_tag}", bufs=sc_bufs,
                              space="PSUM") as psc, \
                 tc.tile_pool(name=f"ps_pv_{aT_tag}", bufs=pv_bufs,
                              space="PSUM") as ppv, \
                 tc.tile_pool(name=f"ex_{aT_tag}", bufs=2) as exp_pool:
                for c in range(CH):
                    for hf in range(FT):
                        cs = slice(NC_ * c, NC_ * (c + 1))
                        ex3 = exp_pool.tile([P, sk_tiles, 2, NC_], F8, tag="ex")
                        for sk in range(sk_tiles):
                            rows = min(P, n_sk - P * sk)
                            sp = psc.tile([P, 2, NC_], F32, tag="sp")
                            for par in range(2):
                                hp = slice(DH * par, DH * par + DH)
                                nc.tensor.matmul(
                                    sp[:rows, par, :],
                                    kT[hp, hf, P * sk:P * sk + rows],
                                    qT[hp, hf, cs], start=True, stop=True)
                            nc.scalar.activation(ex3[:rows, sk, :, :],
                                                 sp[:rows, :, :], AF.Exp,
                                                 scale=SCALE)
                        pv = ppv.tile([DH + 2, 2, NC_], F32, tag="pv")
                        for par in range(2):
                            h = 2 * hf + par
                            if sk_tiles % 2 == 0:
                                for j in range(sk_tiles // 2):
                                    nc.tensor.matmul(
                                        pv[:, par, :],
                                        v_sb[:, 2 * j:2 * j + 2, h, :],
                                        ex3[:, 2 * j:2 * j + 2, par, :],
                                        start=(j == 0),
                                        stop=(j == sk_tiles // 2 - 1),
                                        perf_mode=DR)
                            else:
                                for sk in range(sk_tiles):
                                    rows = min(P, n_sk - P * sk)
                                    nc.tensor.matmul(
                                        pv[:, par, :], v_sb[:rows, sk, h, :],
                                        ex3[:rows, sk, par, :],
                                        start=(sk == 0), stop=(sk == sk_tiles - 1))
                        rc = rowp.tile([1, 2, NC_], F32, tag="rc")
                        nc.vector.reciprocal(rc[:], pv[DH:DH + 1, :, :])
                        for par in range(2):
                            hp = slice(DH * par, DH * par + DH)
                            den_b = bcp.tile([DH, NC_], F32, tag="denb")
                            nc.gpsimd.partition_broadcast(den_b[:], rc[0:1, par, :])
                            nc.vector.tensor_tensor(aT[hp, hf, cs],
                                                    pv[0:DH, par, :],
                                                    den_b[:], ALU.mult)
                        if hook is not None:
                            hook(c * FT + hf)
                    if chunk_cb is not None:
                        chunk_cb(c)

        def out_proj_chunk(ops, w_r, bias_c, aT, src, dst, c):
            """dst[:, :, chunk c] = src + (W*WS).T @ aT / WS + bias."""
            cs = slice(NC_ * c, NC_ * (c + 1))
            for mo in range(FT):
                pt = ops.tile([P, NC_], F32, tag="t")
                for j in range(FT // 2):
                    nc.tensor.matmul(
                        pt[:], w_r[:, 2 * j:2 * j + 2, P * mo:P * (mo + 1)],
                        aT[:, 2 * j:2 * j + 2, cs],
                        start=(j == 0), stop=(j == FT // 2 - 1),
                        perf_mode=DR)
                t1 = tmp.tile([P, NC_], F32R, tag="epi")
                nc.vector.tensor_scalar(
                    t1[:], pt[:], WSI, bias_c[:, mo:mo + 1],
                    op0=ALU.mult, op1=ALU.add)
                nc.gpsimd.tensor_add(dst[:, mo, cs], t1[:], src[:, mo, cs])

        # ================= self-attention =================
        wpool = es.enter_context(tc.tile_pool(name="wpool", bufs=1))
        with tc.tile_pool(name="a1", bufs=1) as a1:
            h1 = layer_norm(xT, b1c, a1, "h1")
            wq1r = load_w8(wpool, "wq1", FT, "wq1r", nc.scalar, scale_col=g1ws)
            wk1r = load_w8(wpool, "wk1", FT, "wk1r", nc.scalar, scale_col=g1ws)
            wv1r = load_w8(wpool, "wv1", FT, "wv1r", nc.scalar, scale_col=g1ws)
            # attn2 enc-side weights: convert on ACT before the exp phase starts
            wk2r = load_w8(wpool, "wk2", KE, "wk2r", nc.scalar)
            wv2r = load_w8(wpool, "wv2", KE, "wv2r", nc.scalar)
            qT1 = project_dr(wq1r, h1, a1, "qT1")
            kT1 = project_dr(wk1r, h1, a1, "kT1")
            v1 = a1.tile([P, ST, NH, DH + 2], F8, tag="v1")
            nc.vector.tensor_copy(
                v1[:, :, :, DH:DH + 1],
                ones_f[:, 0:ST * NH].rearrange("p (a b c) -> p a b c",
                                               a=ST, b=NH))
            nc.vector.tensor_copy(
                v1[:, :, :, DH + 1:DH + 2],
                zeros_f[:, 0:ST * NH].rearrange("p (a b c) -> p a b c",
                                                a=ST, b=NH))
            with tc.tile_pool(name="ps_v1", bufs=2, space="PSUM") as vps:
                for st in range(ST):
                    pt = vps.tile([P, NC_], F32, tag="t")
                    for j in range(FT // 2):
                        nc.tensor.matmul(
                            pt[:], h1[:, 2 * j:2 * j + 2, P * st:P * (st + 1)],
                            wv1r[:, 2 * j:2 * j + 2, :],
                            start=(j == 0), stop=(j == FT // 2 - 1),
                            perf_mode=DR)
                    nc.vector.tensor_scalar_mul(
                        v1[:, st, :, 0:DH],
                        pt[:].rearrange("p (h d) -> p h d", h=NH), WSI)

            wo1r = load_w8(wpool, "wo1", FT, "wo1r", nc.scalar)
            wq2r = load_w8(wpool, "wq2", FT, "wq2r", nc.scalar, scale_col=g2ws)
            wo2r = load_w8(wpool, "wo2", FT, "wo2r", nc.vector)

            # kT2/v2 depend only on enc: compute during attn1
            kT2 = wpool.tile([P, FT, EPAD], F8, tag="kT2")
            v2 = wpool.tile([SK2, 1, NH, DH + 2], F8, tag="v2")
            nc.vector.tensor_copy(
                v2[:, :, :, DH:DH + 1],
                ones_f[0:SK2, 0:NH].rearrange("p (a b c) -> p a b c", a=1, b=NH))
            nc.vector.tensor_copy(
                v2[:, :, :, DH + 1:DH + 2],
                zeros_f[0:SK2, 0:NH].rearrange("p (a b c) -> p a b c", a=1, b=NH))
            with tc.tile_pool(name="ps_kv2", bufs=2, space="PSUM") as kvps:
                for mo in range(FT):
                    pt = kvps.tile([P, NC_], F32, tag="t")
                    for j in range(KE // 2):
                        nc.tensor.matmul(
                            pt[:, 0:EPAD],
                            wk2r[:, 2 * j:2 * j + 2, P * mo:P * (mo + 1)],
                            encT[:, 2 * j:2 * j + 2, :],
                            start=(j == 0), stop=(j == KE // 2 - 1),
                            perf_mode=DR)
                    nc.scalar.mul(kT2[:, mo, :], pt[:, 0:EPAD], WSI)
                pt = kvps.tile([P, NC_], F32, tag="t")
                for j in range(KE // 2):
                    nc.tensor.matmul(pt[:, :], encT[:, 2 * j:2 * j + 2, :],
                                     wv2r[:, 2 * j:2 * j + 2, :],
                                     start=(j == 0), stop=(j == KE // 2 - 1),
                                     perf_mode=DR)
                nc.scalar.mul(
                    v2[:, 0, :, 0:DH],
                    pt[0:SK2, :].rearrange("p (h d) -> p h d", h=NH), WSI)

            # wg streamed + converted on Pool while attn1's ACT runs exp
            wgur = wpool.tile([P, NI, FT, P], F8, tag="wgur")
            wggr = wpool.tile([P, NI, FT, P], F8, tag="wggr")
            wg_r = w_in["wg"].rearrange("(ko ki) n -> ki ko n", ki=P)
            ws_g3 = const.tile([P, FT, P], F32, tag="ws_g3")
            for ft in range(FT):
                nc.vector.tensor_scalar(
                    ws_g3[:, ft, :], ones_f[:, 0:P], g3c[:, ft:ft + 1], WS,
                    op0=ALU.mult, op1=ALU.mult)
            ws_v = ws_g3[:]

            def wg_hook(slot):
                for i in range(2 * slot, 2 * slot + 2):
                    wgu = wgstg.tile([P, FT, P], F32, tag="wgst")
                    nc.sync.dma_start(wgu[:], wg_r[:, :, P * i:P * (i + 1)])
                    nc.gpsimd.tensor_tensor(wgur[:, i, :, :], wgu[:], ws_v,
                                            ALU.mult)
                    wgg = wgstg.tile([P, FT, P], F32, tag="wgst")
                    nc.sync.dma_start(
                        wgg[:], wg_r[:, :, FF + P * i:FF + P * (i + 1)])
                    nc.gpsimd.tensor_tensor(wggr[:, i, :, :], wgg[:], ws_v,
                                            ALU.mult)

            xT1 = resid.tile([P, FT, S], F32R, tag="x")
            h2 = actp.tile([P, FT, S], F8, tag="h2")
            aT1_box = []

            def o1_cb(c):
                out_proj_chunk(ops1, wo1r, bo1c, aT1_box[0], xT, xT1, c)
                ln_chunk(xT1, b2c, h2, ops1, c)

            with tc.tile_pool(name="ps_o1", bufs=1, space="PSUM") as ops1:
                aT1_box.append(a1.tile([P, FT, S], F8, tag="aT1", name="aT1"))
                attention_into(qT1, kT1, v1, S, aT1_box[0], "aT1", hook=wg_hook,
                               chunk_cb=o1_cb)

        # ================= cross-attention =================
        with tc.tile_pool(name="a2", bufs=1) as a2:
            qT2 = project_dr(wq2r, h2, a2, "qT2", copy_eng="act")

            # wf streamed + converted on Pool during attn2
            wfr = wpool.tile([P, NI, D], F8, tag="wfr")
            wf_r = w_in["wf"].rearrange("(ko ki) n -> ki ko n", ki=P)

            def wf_hook(slot):
                if slot >= FT:
                    return
                kq = slot
                wf_stg = wfstg.tile([P, FT, D], F32, tag="wfst")
                nc.sync.dma_start(wf_stg[:], wf_r[:, FT * kq:FT * (kq + 1), :])
                for ko in range(FT):
                    nc.gpsimd.tensor_tensor(
                        wfr[:, FT * kq + ko, :], wf_stg[:, ko, :], ws_t[:],
                        ALU.mult)

            xT2 = resid.tile([P, FT, S], F32R, tag="x")
            h3 = actp.tile([P, FT, S], F8, tag="h3")
            aT2_box = []

            def o2_cb(c):
                out_proj_chunk(ops2, wo2r, bo2c, aT2_box[0], xT1, xT2, c)
                ln_chunk(xT2, b3c, h3, ops2, c)

            with tc.tile_pool(name="ps_o2", bufs=1, space="PSUM") as ops2:
                aT2_box.append(a2.tile([P, FT, S], F8, tag="aT2", name="aT2"))
                attention_into(qT2, kT2, v2, SK2, aT2_box[0], "aT2", hook=wf_hook,
                               chunk_cb=o2_cb, sc_bufs=1, pv_bufs=2)

        # ================= GEGLU feed-forward =================
        with tc.tile_pool(name="ffp", bufs=1) as ffp, \
             tc.tile_pool(name="fft_p", bufs=4) as fft_p, \
             tc.tile_pool(name="gel_p", bufs=4) as gel_p:
            xT3 = resid.tile([P, FT, S], F32R, tag="x")
            with tc.tile_pool(name="ps_ff", bufs=4, space="PSUM") as ffps, \
                 tc.tile_pool(name="ps_wf", bufs=4, space="PSUM") as wfps:
                for c in range(CH):
                    cs = slice(NC_ * c, NC_ * (c + 1))
                    wf_ps = [wfps.tile([P, NC_], F32, tag="a",
                                       name=f"wf_ps{c}_{m}") for m in range(FT)]
                    for i in range(NI):
                        pu = ffps.tile([P, NC_], F32, tag="t")
                        for j in range(FT // 2):
                            nc.tensor.matmul(pu[:],
                                             wgur[:, i, 2 * j:2 * j + 2, :],
                                             h3[:, 2 * j:2 * j + 2, cs],
                                             start=(j == 0), stop=(j == FT // 2 - 1),
                                             perf_mode=DR)
                        pg = ffps.tile([P, NC_], F32, tag="t")
                        for j in range(FT // 2):
                            nc.tensor.matmul(pg[:],
                                             wggr[:, i, 2 * j:2 * j + 2, :],
                                             h3[:, 2 * j:2 * j + 2, cs],
                                             start=(j == 0), stop=(j == FT // 2 - 1),
                                             perf_mode=DR)
                        gel = gel_p.tile([P, NC_], BF16, tag="gel")
                        nc.scalar.activation(gel[:], pg[:], AF.Gelu,
                                             bias=bgc[:, NI + i:NI + i + 1],
                                             scale=WSI)
                        if i % 2 == 0:
                            fpair = fft_p.tile([P, 2, NC_], F8, tag="fpair")
                        # fft = (pu + 32*bg_u) * gel = 32 * (u+bu)*gelu(g+bgg)
                        nc.vector.scalar_tensor_tensor(
                            fpair[:, i % 2, :], pu[:], bguc[:, i:i + 1], gel[:],
                            op0=ALU.add, op1=ALU.mult)
                        if i % 2 == 1:
                            for m in range(FT):
                                nc.tensor.matmul(
                                    wf_ps[m][:],
                                    wfr[:, i - 1:i + 1, P * m:P * (m + 1)],
                                    fpair[:], start=(i == 1), stop=(i == NI - 1),
                                    perf_mode=DR, skip_group_check=True)
                    for m in range(FT):
                        t1 = tmp.tile([P, NC_], F32R, tag="epi")
                        nc.vector.tensor_scalar(
                            t1[:], wf_ps[m][:], WSI * WSI, bfc[:, m:m + 1],
                            op0=ALU.mult, op1=ALU.add)
                        nc.gpsimd.tensor_add(xT3[:, m, cs], t1[:], xT2[:, m, cs])
                    # transpose back & store this chunk immediately
                    for st in range(4 * c, 4 * c + 4):
                        ot = stage.tile([P, D], F32, tag="x_raw")
                        pt = ffps.tile([P, NC_], F32R, tag="t")
                        for ft in range(FT):
                            nc.tensor.transpose(pt[:, P * ft:P * (ft + 1)],
                                                xT3[:, ft, P * st:P * (st + 1)],
                                                ident_r[:])
                        nc.vector.tensor_copy(ot[:], pt[:])
                        nc.sync.dma_start(out[P * st:P * (st + 1), :], ot[:])

    return nc


_CACHED = {}


def _get_nc():
    if "nc" not in _CACHED:
        nc = bacc.Bacc("TRN2", target_bir_lowering=False, debug=False, num_devices=B)
        build(nc)
        nc.finalize()
        _CACHED["nc"] = nc
    return _CACHED["nc"]


def kernel(**inputs) -> np.ndarray:
    nc = _get_nc()
    x = np.ascontiguousarray(np.asarray(inputs["x"]), dtype=np.float32)
    enc = np.ascontiguousarray(np.asarray(inputs["enc"]), dtype=np.float32)
    shared = {k: np.ascontiguousarray(np.asarray(v), dtype=np.float32)
              for k, v in inputs.items() if k not in ("x", "enc")}
    in_maps = [dict(shared, x=x[i], enc=enc[i]) for i in range(B)]
    res = run_bass_kernel_spmd(nc, in_maps, core_ids=list(range(B)))
    outs = []
    for i in range(B):
        o = np.asarray(res.results[i]["out"])
        outs.append(o.view(np.float32) if o.dtype != np.float32 else o)
    return np.stack(outs, axis=0)


if __name__ == "__main__":
    print("module import ok")


# revision 65
# speedup vs baseline: 1.0146x; 1.0036x over previous
"""Trainium2 Bass kernel for a BasicTransformerBlock (self-attn + cross-attn + GEGLU FF).

Sharding: data-parallel over the batch axis — 8 batch elements onto 8 NeuronCores,
same SPMD program, no collectives.

Design (driven by the TRN2 timeline cost model):
- Residual stream stays TRANSPOSED in SBUF as xT[d, s] (features on partitions,
  f32r), so every linear runs straight off the HBM weight layout; only the
  kernel entry/exit transpose via the PE (f32r identity).
- Heavy matmuls run in fp8e4 with DoubleRow perf mode: one instruction
  contracts TWO 128-row k-tiles at 0.5 cycles per moving element (4x fp32r).
  Weights are scaled by 32 on the f32->fp8 convert (sigma-0.02 weights would
  hit fp8 subnormals); the 1/32 descale rides the PSUM->SBUF copies that must
  exist anyway. LN gains g are folded into the same converts for free, so the
  LayerNorm itself only subtracts mu, multiplies rstd and adds b — three fused
  [128, 4, 512] DVE passes using zero-stride broadcast APs.
- Attention: scores in plain fp8 (K=64/head); exp reads score-PSUM pairs
  [128, 2, 512] in one ACT instruction and writes probs directly in fp8;
  probs@v uses DoubleRow over paired source-token tiles with the softmax
  denominator riding as a ones column of v (pv row 64); one reciprocal per
  (chunk, head-pair) on a par-indexed two-bank pv tile.
- Engines have in-order queues, so emission order is the schedule: wg/wf
  weight DMA+convert pairs are interleaved into the attention loops (attn1 is
  exp/ACT-bound for ~60us, so Pool does the converts there); kT2/v2 are
  computed during attn1; out-projections and the NEXT LayerNorm are emitted
  per-chunk from inside the attention loop (chunk_cb) to pipeline phases.
"""
import sys

sys.path.insert(0, "/opt/trn_rl_repo")

from contextlib import ExitStack

import numpy as np

import concourse.bass as bass
import concourse.mybir as mybir
import concourse.tile as tile
from concourse import bacc
from concourse.bass_utils import run_bass_kernel_spmd
from concourse.masks import make_identity

F32 = mybir.dt.float32
F32R = mybir.dt.float32r
BF16 = mybir.dt.bfloat16
F8 = mybir.dt.float8e4
AF = mybir.ActivationFunctionType
ALU = mybir.AluOpType
DR = mybir.MatmulPerfMode.DoubleRow

B = 8
S = 1024          # tokens
D = 512           # model dim
SK2 = 77          # cross-attention source length
DE = 768          # encoder dim
FF = 2048         # GEGLU inner dim (per half)
NH = 8            # heads
DH = 64           # head dim
SCALE = DH ** -0.5
EPS = 1e-5
P = 128
NC_ = 512         # token chunk (one psum bank of fp32)
ST = S // P       # 8 token tiles
FT = D // P       # 4 feature tiles
CH = S // NC_     # 2 token chunks
KE = DE // P      # 6 encoder feature tiles
NI = FF // P      # 16 FF inner tiles
WS = 32.0         # fp8 weight scale (keeps sigma~0.02 weights out of subnormals)
WSI = 1.0 / WS
EPAD = 128        # padded encT row pitch


def build(nc: bass.Bass):
    x = nc.dram_tensor("x", [S, D], F32, kind="ExternalInput")
    enc = nc.dram_tensor("enc", [SK2, DE], F32, kind="ExternalInput")
    w_in = {}
    for name, shape in [
        ("wq1", [D, D]), ("wk1", [D, D]), ("wv1", [D, D]), ("wo1", [D, D]),
        ("wq2", [D, D]), ("wk2", [DE, D]), ("wv2", [DE, D]), ("wo2", [D, D]),
        ("wg", [D, 2 * FF]), ("wf", [FF, D]),
    ]:
        w_in[name] = nc.dram_tensor(name, shape, F32, kind="ExternalInput")
    vec_in = {}
    for name, n in [("ln1_g", D), ("ln1_b", D), ("ln2_g", D), ("ln2_b", D),
                    ("ln3_g", D), ("ln3_b", D), ("bo1", D), ("bo2", D),
                    ("bg", 2 * FF), ("bf", D)]:
        vec_in[name] = nc.dram_tensor(name, [n], F32, kind="ExternalInput")
    out = nc.dram_tensor("out", [S, D], F32, kind="ExternalOutput")

    with tile.TileContext(nc) as tc, ExitStack() as es:
        const = es.enter_context(tc.tile_pool(name="const", bufs=1))
        resid = es.enter_context(tc.tile_pool(name="resid", bufs=2))
        stage = es.enter_context(tc.tile_pool(name="stage", bufs=2))
        wgstg = es.enter_context(tc.tile_pool(name="wgstg", bufs=4))
        wfstg = es.enter_context(tc.tile_pool(name="wfstg", bufs=1))
        rowp = es.enter_context(tc.tile_pool(name="rowp", bufs=1))
        bcp = es.enter_context(tc.tile_pool(name="bcp", bufs=2))
        tmp = es.enter_context(tc.tile_pool(name="tmp", bufs=2))
        lnt = es.enter_context(tc.tile_pool(name="lnt", bufs=1))
        actp = es.enter_context(tc.tile_pool(name="actp", bufs=1))

        # ---- constants ----
        ident_f = const.tile([P, P], F32)
        make_identity(nc, ident_f[:])
        ones_f = const.tile([P, P], F32)
        nc.vector.memset(ones_f[:], 1.0)
        zeros_f = const.tile([P, P], F32)
        nc.vector.memset(zeros_f[:], 0.0)
        ones128 = const.tile([P, 1], F32R)         # stats lhsT (K=128, M=1)
        nc.vector.tensor_copy(ones128[:], ones_f[:, 0:1])
        eps_t = const.tile([1, 1], F32)
        nc.vector.memset(eps_t[:], EPS)
        ws_t = const.tile([P, NC_], F32)           # x32 tile for Pool converts
        nc.vector.memset(ws_t[:], WS)
        ident_r = const.tile([P, P], F32R)
        nc.vector.tensor_copy(ident_r[:], ident_f[:])

        def col_const(name, n):
            t = const.tile([P, n], F32, tag=f"{name}_c")
            nc.sync.dma_start(t[:], vec_in[name].rearrange("(o p) -> p o", p=P))
            return t

        g1c, b1c = col_const("ln1_g", FT), col_const("ln1_b", FT)
        g2c, b2c = col_const("ln2_g", FT), col_const("ln2_b", FT)
        g3c, b3c = col_const("ln3_g", FT), col_const("ln3_b", FT)
        bo1c, bo2c = col_const("bo1", FT), col_const("bo2", FT)
        bfc = col_const("bf", FT)
        bgc = col_const("bg", 2 * NI)   # [:, 0:16]=u biases, [:, 16:32]=g biases
        # u-side bias prescaled by WS (descale happens at the wf epilogue)
        bguc = const.tile([P, NI], F32, tag="bgu32")
        nc.vector.tensor_scalar_mul(bguc[:], bgc[:, 0:NI], WS)
        g1ws = const.tile([P, FT], F32, tag="g1ws")
        nc.vector.tensor_scalar_mul(g1ws[:], g1c[:], WS)
        g2ws = const.tile([P, FT], F32, tag="g2ws")
        nc.vector.tensor_scalar_mul(g2ws[:], g2c[:], WS)

        # ---- load x, PE-transpose into xT [128, FT, S] (f32r) ----
        xT = resid.tile([P, FT, S], F32R, tag="x")
        encT = const.tile([P, KE, EPAD], F8, tag="encT")
        with tc.tile_pool(name="ps_in", bufs=2, space="PSUM") as ps_in, \
             tc.tile_pool(name="encstg", bufs=1) as encstg:
            for st in range(ST):
                xr = stage.tile([P, D], F32, tag="x_raw")
                nc.sync.dma_start(xr[:], x[P * st:P * (st + 1), :])
                pt = ps_in.tile([P, NC_], F32, tag="t")
                for ft in range(FT):
                    nc.tensor.transpose(pt[:, P * ft:P * (ft + 1)],
                                        xr[:, P * ft:P * (ft + 1)], ident_f[:])
                nc.scalar.copy(
                    xT[:, :, P * st:P * (st + 1)],
                    pt[:].rearrange("p (f q) -> p f q", f=FT))

            # ---- enc: PE transposes into encT fp8 (padded pitch) ----
            enc_raw = encstg.tile([SK2, DE], F32, tag="enc_raw")
            nc.sync.dma_start(enc_raw[:], enc[:, :])
            for ke in range(KE):
                pt = ps_in.tile([P, NC_], F32, tag="t")
                nc.tensor.transpose(pt[:, 0:SK2],
                                    enc_raw[:, P * ke:P * (ke + 1)],
                                    ident_f[0:SK2, 0:SK2])
                nc.vector.tensor_copy(encT[:, ke, SK2:EPAD],
                                      zeros_f[:, 0:EPAD - SK2])
                nc.vector.tensor_copy(encT[:, ke, 0:SK2], pt[:, 0:SK2])

        def load_w8(pool, name, kouter, tag, eng, gcol=None, scale_col=None):
            """Stream a [K, N<=512] HBM weight into [128, kouter, N] fp8 (xWS),
            optionally folding a per-input-feature LN gain g into the rows
            (scale_col = g*WS columns, applied via the ACT scale pointer)."""
            dram = w_in[name]
            nout = dram.shape[1]
            wr = pool.tile([P, kouter, nout], F8, tag=tag)
            dram_r = dram.rearrange("(ko ki) n -> ki ko n", ki=P)
            half = (kouter + 1) // 2 if kouter > 4 else kouter
            for k0 in range(0, kouter, half):
                k1 = min(k0 + half, kouter)
                stg = stage.tile([P, half, nout], F32, tag="wst")
                nc.sync.dma_start(stg[:, 0:k1 - k0, :], dram_r[:, k0:k1, :])
                if scale_col is not None:
                    for ko in range(k0, k1):
                        nc.scalar.activation(
                            wr[:, ko, :], stg[:, ko - k0, :], AF.Copy,
                            scale=scale_col[:, ko:ko + 1])
                elif gcol is not None:
                    for ko in range(k0, k1):
                        nc.vector.tensor_scalar(
                            wr[:, ko, :], stg[:, ko - k0, :],
                            gcol[:, ko:ko + 1], WS, op0=ALU.mult, op1=ALU.mult)
                elif eng is nc.scalar:
                    eng.mul(wr[:, k0:k1, :], stg[:, 0:k1 - k0, :], WS)
                else:
                    eng.tensor_scalar_mul(wr[:, k0:k1, :], stg[:, 0:k1 - k0, :], WS)
            return wr

        def ln_chunk(src, bcol, hT, lps, c, st_tag="st"):
            """Emit LayerNorm chunk c: src f32r -> hT fp8 (g folded into the
            consuming weights, +b applied here). Stats share one psum bank."""
            cs = slice(NC_ * c, NC_ * (c + 1))
            st_ps = lps.tile([1, NC_], F32, tag=st_tag)
            for ft in range(FT):
                nc.tensor.matmul(st_ps[:], ones128[:], src[:, ft, cs],
                                 start=(ft == 0), stop=(ft == FT - 1))
            mu = rowp.tile([1, NC_], F32, tag="mu")
            nc.vector.tensor_scalar_mul(mu[:], st_ps[:], 1.0 / D)
            sq_ps = lps.tile([1, NC_], F32, tag=st_tag, name="sqps")
            for ft in range(FT):
                xsq = tmp.tile([P, NC_], F32R, tag="xsq")
                nc.scalar.activation(xsq[:], src[:, ft, cs], AF.Square)
                nc.tensor.matmul(sq_ps[:], ones128[:], xsq[:],
                                 start=(ft == 0), stop=(ft == FT - 1))
            musq = rowp.tile([1, NC_], F32, tag="musq")
            nc.vector.tensor_mul(musq[:], mu[:], mu[:])
            var = rowp.tile([1, NC_], F32, tag="var")
            nc.vector.scalar_tensor_tensor(
                var[:], sq_ps[:], 1.0 / D, musq[:],
                op0=ALU.mult, op1=ALU.subtract)
            sd = rowp.tile([1, NC_], F32, tag="sd")
            nc.scalar.activation(sd[:], var[:], AF.Sqrt, bias=eps_t[:])
            rstd = rowp.tile([1, NC_], F32, tag="rstd")
            nc.vector.reciprocal(rstd[:], sd[:])
            mu_b = bcp.tile([P, NC_], F32, tag="mub")
            nc.gpsimd.partition_broadcast(mu_b[:], mu[:])
            rstd_b = bcp.tile([P, NC_], F32, tag="rstdb")
            nc.gpsimd.partition_broadcast(rstd_b[:], rstd[:])
            t = lnt.tile([P, FT, NC_], F32R, tag="lt")
            nc.vector.tensor_tensor(
                t[:], src[:, :, cs],
                mu_b[:, None, :].broadcast_to([P, FT, NC_]), ALU.subtract)
            nc.vector.tensor_tensor(
                t[:], t[:],
                rstd_b[:, None, :].broadcast_to([P, FT, NC_]), ALU.mult)
            nc.vector.tensor_tensor(
                hT[:, :, cs], t[:],
                bcol[:, :, None].broadcast_to([P, FT, NC_]), ALU.add)

        def layer_norm(src, bcol, pool, tag):
            hT = pool.tile([P, FT, S], F8, tag=tag, name=f"h_{tag}")
            with tc.tile_pool(name=f"ps_{tag}", bufs=2, space="PSUM") as lps:
                for c in range(CH):
                    ln_chunk(src, bcol, hT, lps, c)
            return hT

        def project_dr(w_r, src, pool, tag, copy_eng="dve"):
            """yT = (W*WS).T @ src via fp8 DoubleRow, descaled 1/WS on copy.
            src [128, FT, S] fp8; w_r [128, FT, D] fp8. -> [128, FT, S] fp8."""
            yT = pool.tile([P, FT, S], F8, tag=tag)
            with tc.tile_pool(name=f"ps_{tag}", bufs=2, space="PSUM") as pps:
                for c in range(CH):
                    cs = slice(NC_ * c, NC_ * (c + 1))
                    for mo in range(FT):
                        pt = pps.tile([P, NC_], F32, tag="t")
                        for j in range(FT // 2):
                            nc.tensor.matmul(
                                pt[:], w_r[:, 2 * j:2 * j + 2, P * mo:P * (mo + 1)],
                                src[:, 2 * j:2 * j + 2, cs],
                                start=(j == 0), stop=(j == FT // 2 - 1),
                                perf_mode=DR)
                        if copy_eng == "act":
                            nc.scalar.mul(yT[:, mo, cs], pt[:], WSI)
                        else:
                            nc.vector.tensor_scalar_mul(yT[:, mo, cs], pt[:], WSI)
            return yT

        def attention_into(qT, kT, v_sb, n_sk, aT, aT_tag, hook=None,
                           chunk_cb=None, sc_bufs=2, pv_bufs=1):
            """qT/kT: [128, FT, S*] fp8 transposed; v_sb: [part, sk_tiles, NH,
            DH+2] fp8 with ones in col DH. Fills aT [128, FT, S] fp8.
            hook(slot) emits interleaved filler work; chunk_cb(c) emits the
            per-chunk consumer (out-projection) as soon as chunk c is done."""
            sk_tiles = (n_sk + P - 1) // P
            with tc.tile_pool(name=f"ps_sc_{aT_tag}", bufs=sc_bufs,
                              space="PSUM") as psc, \
                 tc.tile_pool(name=f"ps_pv_{aT_tag}", bufs=pv_bufs,
                              space="PSUM") as ppv, \
                 tc.tile_pool(name=f"ex_{aT_tag}", bufs=2) as exp_pool:
                for c in range(CH):
                    for hf in range(FT):
                        cs = slice(NC_ * c, NC_ * (c + 1))
                        ex3 = exp_pool.tile([P, sk_tiles, 2, NC_], F8, tag="ex")
                        for sk in range(sk_tiles):
                            rows = min(P, n_sk - P * sk)
                            sp = psc.tile([P, 2, NC_], F32, tag="sp")
                            for par in range(2):
                                hp = slice(DH * par, DH * par + DH)
                                nc.tensor.matmul(
                                    sp[:rows, par, :],
                                    kT[hp, hf, P * sk:P * sk + rows],
                                    qT[hp, hf, cs], start=True, stop=True)
                            nc.scalar.activation(ex3[:rows, sk, :, :],
                                                 sp[:rows, :, :], AF.Exp,
                                                 scale=SCALE)
                        pv = ppv.tile([DH + 2, 2, NC_], F32, tag="pv")
                        for par in range(2):
                            h = 2 * hf + par
                            if sk_tiles % 2 == 0:
                                for j in range(sk_tiles // 2):
                                    nc.tensor.matmul(
                                        pv[:, par, :],
                                        v_sb[:, 2 * j:2 * j + 2, h, :],
                                        ex3[:, 2 * j:2 * j + 2, par, :],
                                        start=(j == 0),
                                        stop=(j == sk_tiles // 2 - 1),
                                        perf_mode=DR)
                            else:
                                for sk in range(sk_tiles):
                                    rows = min(P, n_sk - P * sk)
                                    nc.tensor.matmul(
                                        pv[:, par, :], v_sb[:rows, sk, h, :],
                                        ex3[:rows, sk, par, :],
                                        start=(sk == 0), stop=(sk == sk_tiles - 1))
                        rc = rowp.tile([1, 2, NC_], F32, tag="rc")
                        nc.vector.reciprocal(rc[:], pv[DH:DH + 1, :, :])
                        for par in range(2):
                            hp = slice(DH * par, DH * par + DH)
                            den_b = bcp.tile([DH, NC_], F32, tag="denb")
                            nc.gpsimd.partition_broadcast(den_b[:], rc[0:1, par, :])
                            nc.vector.tensor_tensor(aT[hp, hf, cs],
                                                    pv[0:DH, par, :],
                                                    den_b[:], ALU.mult)
                        if hook is not None:
                            hook(c * FT + hf)
                    if chunk_cb is not None:
                        chunk_cb(c)

        def out_proj_chunk(ops, w_r, bias_c, aT, src, dst, c):
            """dst[:, :, chunk c] = src + (W*WS).T @ aT / WS + bias."""
            cs = slice(NC_ * c, NC_ * (c + 1))
            for mo in range(FT):
                pt = ops.tile([P, NC_], F32, tag="t")
                for j in range(FT // 2):
                    nc.tensor.matmul(
                        pt[:], w_r[:, 2 * j:2 * j + 2, P * mo:P * (mo + 1)],
                        aT[:, 2 * j:2 * j + 2, cs],
                        start=(j == 0), stop=(j == FT // 2 - 1),
                        perf_mode=DR)
                t1 = tmp.tile([P, NC_], F32R, tag="epi")
                nc.vector.tensor_scalar(
                    t1[:], pt[:], WSI, bias_c[:, mo:mo + 1],
                    op0=ALU.mult, op1=ALU.add)
                nc.vector.tensor_add(dst[:, mo, cs], t1[:], src[:, mo, cs])

        # ================= self-attention =================
        wpool = es.enter_context(tc.tile_pool(name="wpool", bufs=1))
        with tc.tile_pool(name="a1", bufs=1) as a1:
            h1 = layer_norm(xT, b1c, a1, "h1")
            wq1r = load_w8(wpool, "wq1", FT, "wq1r", nc.scalar, scale_col=g1ws)
            wk1r = load_w8(wpool, "wk1", FT, "wk1r", nc.scalar, scale_col=g1ws)
            wv1r = load_w8(wpool, "wv1", FT, "wv1r", nc.scalar, scale_col=g1ws)
            # attn2 enc-side weights: convert on ACT before the exp phase starts
            wk2r = load_w8(wpool, "wk2", KE, "wk2r", nc.scalar)
            wv2r = load_w8(wpool, "wv2", KE, "wv2r", nc.scalar)
            qT1 = project_dr(wq1r, h1, a1, "qT1")
            kT1 = project_dr(wk1r, h1, a1, "kT1")
            v1 = a1.tile([P, ST, NH, DH + 2], F8, tag="v1")
            nc.vector.tensor_copy(
                v1[:, :, :, DH:DH + 1],
                ones_f[:, 0:ST * NH].rearrange("p (a b c) -> p a b c",
                                               a=ST, b=NH))
            nc.vector.tensor_copy(
                v1[:, :, :, DH + 1:DH + 2],
                zeros_f[:, 0:ST * NH].rearrange("p (a b c) -> p a b c",
                                                a=ST, b=NH))
            with tc.tile_pool(name="ps_v1", bufs=2, space="PSUM") as vps:
                for st in range(ST):
                    pt = vps.tile([P, NC_], F32, tag="t")
                    for j in range(FT // 2):
                        nc.tensor.matmul(
                            pt[:], h1[:, 2 * j:2 * j + 2, P * st:P * (st + 1)],
                            wv1r[:, 2 * j:2 * j + 2, :],
                            start=(j == 0), stop=(j == FT // 2 - 1),
                            perf_mode=DR)
                    nc.vector.tensor_scalar_mul(
                        v1[:, st, :, 0:DH],
                        pt[:].rearrange("p (h d) -> p h d", h=NH), WSI)

            wo1r = load_w8(wpool, "wo1", FT, "wo1r", nc.scalar)
            wq2r = load_w8(wpool, "wq2", FT, "wq2r", nc.scalar, scale_col=g2ws)
            wo2r = load_w8(wpool, "wo2", FT, "wo2r", nc.vector)

            # kT2/v2 depend only on enc: compute during attn1
            kT2 = wpool.tile([P, FT, EPAD], F8, tag="kT2")
            v2 = wpool.tile([SK2, 1, NH, DH + 2], F8, tag="v2")
            nc.vector.tensor_copy(
                v2[:, :, :, DH:DH + 1],
                ones_f[0:SK2, 0:NH].rearrange("p (a b c) -> p a b c", a=1, b=NH))
            nc.vector.tensor_copy(
                v2[:, :, :, DH + 1:DH + 2],
                zeros_f[0:SK2, 0:NH].rearrange("p (a b c) -> p a b c", a=1, b=NH))
            with tc.tile_pool(name="ps_kv2", bufs=2, space="PSUM") as kvps:
                for mo in range(FT):
                    pt = kvps.tile([P, NC_], F32, tag="t")
                    for j in range(KE // 2):
                        nc.tensor.matmul(
                            pt[:, 0:EPAD],
                            wk2r[:, 2 * j:2 * j + 2, P * mo:P * (mo + 1)],
                            encT[:, 2 * j:2 * j + 2, :],
                            start=(j == 0), stop=(j == KE // 2 - 1),
                            perf_mode=DR)
                    nc.scalar.mul(kT2[:, mo, :], pt[:, 0:EPAD], WSI)
                pt = kvps.tile([P, NC_], F32, tag="t")
                for j in range(KE // 2):
                    nc.tensor.matmul(pt[:, :], encT[:, 2 * j:2 * j + 2, :],
                                     wv2r[:, 2 * j:2 * j + 2, :],
                                     start=(j == 0), stop=(j == KE // 2 - 1),
                                     perf_mode=DR)
                nc.scalar.mul(
                    v2[:, 0, :, 0:DH],
                    pt[0:SK2, :].rearrange("p (h d) -> p h d", h=NH), WSI)

            # wg streamed + converted on Pool while attn1's ACT runs exp
            wgur = wpool.tile([P, NI, FT, P], F8, tag="wgur")
            wggr = wpool.tile([P, NI, FT, P], F8, tag="wggr")
            wg_r = w_in["wg"].rearrange("(ko ki) n -> ki ko n", ki=P)
            ws_g3 = const.tile([P, FT, P], F32, tag="ws_g3")
            for ft in range(FT):
                nc.vector.tensor_scalar(
                    ws_g3[:, ft, :], ones_f[:, 0:P], g3c[:, ft:ft + 1], WS,
                    op0=ALU.mult, op1=ALU.mult)
            ws_v = ws_g3[:]

            wg_sched = [[0], [1], [2], [3], [4, 5, 6], [7, 8, 9],
                        [10, 11, 12], [13, 14, 15]]

            def wg_hook(slot):
                for i in wg_sched[slot]:
                    wgu = wgstg.tile([P, FT, P], F32, tag="wgst")
                    nc.sync.dma_start(wgu[:], wg_r[:, :, P * i:P * (i + 1)])
                    nc.gpsimd.tensor_tensor(wgur[:, i, :, :], wgu[:], ws_v,
                                            ALU.mult)
                    wgg = wgstg.tile([P, FT, P], F32, tag="wgst")
                    nc.sync.dma_start(
                        wgg[:], wg_r[:, :, FF + P * i:FF + P * (i + 1)])
                    nc.gpsimd.tensor_tensor(wggr[:, i, :, :], wgg[:], ws_v,
                                            ALU.mult)

            xT1 = resid.tile([P, FT, S], F32R, tag="x")
            h2 = actp.tile([P, FT, S], F8, tag="h2")
            aT1_box = []

            def o1_cb(c):
                out_proj_chunk(ops1, wo1r, bo1c, aT1_box[0], xT, xT1, c)
                ln_chunk(xT1, b2c, h2, ops1, c)

            with tc.tile_pool(name="ps_o1", bufs=1, space="PSUM") as ops1:
                aT1_box.append(a1.tile([P, FT, S], F8, tag="aT1", name="aT1"))
                attention_into(qT1, kT1, v1, S, aT1_box[0], "aT1", hook=wg_hook,
                               chunk_cb=o1_cb)

        # ================= cross-attention =================
        with tc.tile_pool(name="a2", bufs=1) as a2:
            qT2 = project_dr(wq2r, h2, a2, "qT2", copy_eng="act")

            # wf streamed + converted on Pool during attn2
            wfr = wpool.tile([P, NI, D], F8, tag="wfr")
            wf_r = w_in["wf"].rearrange("(ko ki) n -> ki ko n", ki=P)

            def wf_hook(slot):
                if slot >= FT:
                    return
                kq = slot
                wf_stg = wfstg.tile([P, FT, D], F32, tag="wfst")
                nc.sync.dma_start(wf_stg[:], wf_r[:, FT * kq:FT * (kq + 1), :])
                for ko in range(FT):
                    nc.gpsimd.tensor_tensor(
                        wfr[:, FT * kq + ko, :], wf_stg[:, ko, :], ws_t[:],
                        ALU.mult)

            xT2 = resid.tile([P, FT, S], F32R, tag="x")
            h3 = actp.tile([P, FT, S], F8, tag="h3")
            aT2_box = []

            def o2_cb(c):
                out_proj_chunk(ops2, wo2r, bo2c, aT2_box[0], xT1, xT2, c)
                ln_chunk(xT2, b3c, h3, ops2, c)

            with tc.tile_pool(name="ps_o2", bufs=1, space="PSUM") as ops2:
                aT2_box.append(a2.tile([P, FT, S], F8, tag="aT2", name="aT2"))
                attention_into(qT2, kT2, v2, SK2, aT2_box[0], "aT2", hook=wf_hook,
                               chunk_cb=o2_cb, sc_bufs=1, pv_bufs=2)

        # ================= GEGLU feed-forward =================
        with tc.tile_pool(name="ffp", bufs=1) as ffp, \
             tc.tile_pool(name="fft_p", bufs=4) as fft_p, \
             tc.tile_pool(name="gel_p", bufs=4) as gel_p:
            xT3 = resid.tile([P, FT, S], F32R, tag="x")
            with tc.tile_pool(name="ps_ff", bufs=4, space="PSUM") as ffps, \
                 tc.tile_pool(name="ps_wf", bufs=4, space="PSUM") as wfps:
                for c in range(CH):
                    cs = slice(NC_ * c, NC_ * (c + 1))
                    wf_ps = [wfps.tile([P, NC_], F32, tag="a",
                                       name=f"wf_ps{c}_{m}") for m in range(FT)]
                    for i in range(NI):
                        pu = ffps.tile([P, NC_], F32, tag="t")
                        for j in range(FT // 2):
                            nc.tensor.matmul(pu[:],
                                             wgur[:, i, 2 * j:2 * j + 2, :],
                                             h3[:, 2 * j:2 * j + 2, cs],
                                             start=(j == 0), stop=(j == FT // 2 - 1),
                                             perf_mode=DR)
                        pg = ffps.tile([P, NC_], F32, tag="t")
                        for j in range(FT // 2):
                            nc.tensor.matmul(pg[:],
                                             wggr[:, i, 2 * j:2 * j + 2, :],
                                             h3[:, 2 * j:2 * j + 2, cs],
                                             start=(j == 0), stop=(j == FT // 2 - 1),
                                             perf_mode=DR)
                        gel = gel_p.tile([P, NC_], BF16, tag="gel")
                        nc.scalar.activation(gel[:], pg[:], AF.Gelu,
                                             bias=bgc[:, NI + i:NI + i + 1],
                                             scale=WSI)
                        if i % 2 == 0:
                            fpair = fft_p.tile([P, 2, NC_], F8, tag="fpair")
                        # fft = (pu + 32*bg_u) * gel = 32 * (u+bu)*gelu(g+bgg)
                        nc.vector.scalar_tensor_tensor(
                            fpair[:, i % 2, :], pu[:], bguc[:, i:i + 1], gel[:],
                            op0=ALU.add, op1=ALU.mult)
                        if i % 2 == 1:
                            for m in range(FT):
                                nc.tensor.matmul(
                                    wf_ps[m][:],
                                    wfr[:, i - 1:i + 1, P * m:P * (m + 1)],
                                    fpair[:], start=(i == 1), stop=(i == NI - 1),
                                    perf_mode=DR, skip_group_check=True)
                    for m in range(FT):
                        t1 = tmp.tile([P, NC_], F32R, tag="epi")
                        nc.vector.tensor_scalar(
                            t1[:], wf_ps[m][:], WSI * WSI, bfc[:, m:m + 1],
                            op0=ALU.mult, op1=ALU.add)
                        nc.vector.tensor_add(xT3[:, m, cs], t1[:], xT2[:, m, cs])
                    # transpose back & store this chunk immediately
                    for st in range(4 * c, 4 * c + 4):
                        ot = stage.tile([P, D], F32, tag="x_raw")
                        pt = ffps.tile([P, NC_], F32R, tag="t")
                        for ft in range(FT):
                            nc.tensor.transpose(pt[:, P * ft:P * (ft + 1)],
                                                xT3[:, ft, P * st:P * (st + 1)],
                                                ident_r[:])
                        nc.vector.tensor_copy(ot[:], pt[:])
                        nc.sync.dma_start(out[P * st:P * (st + 1), :], ot[:])

    return nc


_CACHED = {}


def _get_nc():
    if "nc" not in _CACHED:
        nc = bacc.Bacc("TRN2", target_bir_lowering=False, debug=False, num_devices=B)
        build(nc)
        nc.finalize()
        _CACHED["nc"] = nc
    return _CACHED["nc"]


def kernel(**inputs) -> np.ndarray:
    nc = _get_nc()
    x = np.ascontiguousarray(np.asarray(inputs["x"]), dtype=np.float32)
    enc = np.ascontiguousarray(np.asarray(inputs["enc"]), dtype=np.float32)
    shared = {k: np.ascontiguousarray(np.asarray(v), dtype=np.float32)
              for k, v in inputs.items() if k not in ("x", "enc")}
    in_maps = [dict(shared, x=x[i], enc=enc[i]) for i in range(B)]
    res = run_bass_kernel_spmd(nc, in_maps, core_ids=list(range(B)))
    outs = []
    for i in range(B):
        o = np.asarray(res.results[i]["out"])
        outs.append(o.view(np.float32) if o.dtype != np.float32 else o)
    return np.stack(outs, axis=0)


if __name__ == "__main__":
    print("module import ok")


# revision 66
# speedup vs baseline: 1.0152x; 1.0005x over previous
"""Trainium2 Bass kernel for a BasicTransformerBlock (self-attn + cross-attn + GEGLU FF).

Sharding: data-parallel over the batch axis — 8 batch elements onto 8 NeuronCores,
same SPMD program, no collectives.

Design (driven by the TRN2 timeline cost model):
- Residual stream stays TRANSPOSED in SBUF as xT[d, s] (features on partitions,
  f32r), so every linear runs straight off the HBM weight layout; only the
  kernel entry/exit transpose via the PE (f32r identity).
- Heavy matmuls run in fp8e4 with DoubleRow perf mode: one instruction
  contracts TWO 128-row k-tiles at 0.5 cycles per moving element (4x fp32r).
  Weights are scaled by 32 on the f32->fp8 convert (sigma-0.02 weights would
  hit fp8 subnormals); the 1/32 descale rides the PSUM->SBUF copies that must
  exist anyway. LN gains g are folded into the same converts for free, so the
  LayerNorm itself only subtracts mu, multiplies rstd and adds b — three fused
  [128, 4, 512] DVE passes using zero-stride broadcast APs.
- Attention: scores in plain fp8 (K=64/head); exp reads score-PSUM pairs
  [128, 2, 512] in one ACT instruction and writes probs directly in fp8;
  probs@v uses DoubleRow over paired source-token tiles with the softmax
  denominator riding as a ones column of v (pv row 64); one reciprocal per
  (chunk, head-pair) on a par-indexed two-bank pv tile.
- Engines have in-order queues, so emission order is the schedule: wg/wf
  weight DMA+convert pairs are interleaved into the attention loops (attn1 is
  exp/ACT-bound for ~60us, so Pool does the converts there); kT2/v2 are
  computed during attn1; out-projections and the NEXT LayerNorm are emitted
  per-chunk from inside the attention loop (chunk_cb) to pipeline phases.
"""
import sys

sys.path.insert(0, "/opt/trn_rl_repo")

from contextlib import ExitStack

import numpy as np

import concourse.bass as bass
import concourse.mybir as mybir
import concourse.tile as tile
from concourse import bacc
from concourse.bass_utils import run_bass_kernel_spmd
from concourse.masks import make_identity

F32 = mybir.dt.float32
F32R = mybir.dt.float32r
BF16 = mybir.dt.bfloat16
F8 = mybir.dt.float8e4
AF = mybir.ActivationFunctionType
ALU = mybir.AluOpType
DR = mybir.MatmulPerfMode.DoubleRow

B = 8
S = 1024          # tokens
D = 512           # model dim
SK2 = 77          # cross-attention source length
DE = 768          # encoder dim
FF = 2048         # GEGLU inner dim (per half)
NH = 8            # heads
DH = 64           # head dim
SCALE = DH ** -0.5
EPS = 1e-5
P = 128
NC_ = 512         # token chunk (one psum bank of fp32)
ST = S // P       # 8 token tiles
FT = D // P       # 4 feature tiles
CH = S // NC_     # 2 token chunks
KE = DE // P      # 6 encoder feature tiles
NI = FF // P      # 16 FF inner tiles
WS = 32.0         # fp8 weight scale (keeps sigma~0.02 weights out of subnormals)
WSI = 1.0 / WS
EPAD = 128        # padded encT row pitch


def build(nc: bass.Bass):
    x = nc.dram_tensor("x", [S, D], F32, kind="ExternalInput")
    enc = nc.dram_tensor("enc", [SK2, DE], F32, kind="ExternalInput")
    w_in = {}
    for name, shape in [
        ("wq1", [D, D]), ("wk1", [D, D]), ("wv1", [D, D]), ("wo1", [D, D]),
        ("wq2", [D, D]), ("wk2", [DE, D]), ("wv2", [DE, D]), ("wo2", [D, D]),
        ("wg", [D, 2 * FF]), ("wf", [FF, D]),
    ]:
        w_in[name] = nc.dram_tensor(name, shape, F32, kind="ExternalInput")
    vec_in = {}
    for name, n in [("ln1_g", D), ("ln1_b", D), ("ln2_g", D), ("ln2_b", D),
                    ("ln3_g", D), ("ln3_b", D), ("bo1", D), ("bo2", D),
                    ("bg", 2 * FF), ("bf", D)]:
        vec_in[name] = nc.dram_tensor(name, [n], F32, kind="ExternalInput")
    out = nc.dram_tensor("out", [S, D], F32, kind="ExternalOutput")

    with tile.TileContext(nc) as tc, ExitStack() as es:
        const = es.enter_context(tc.tile_pool(name="const", bufs=1))
        resid = es.enter_context(tc.tile_pool(name="resid", bufs=2))
        stage = es.enter_context(tc.tile_pool(name="stage", bufs=2))
        wgstg = es.enter_context(tc.tile_pool(name="wgstg", bufs=4))
        wfstg = es.enter_context(tc.tile_pool(name="wfstg", bufs=1))
        rowp = es.enter_context(tc.tile_pool(name="rowp", bufs=1))
        bcp = es.enter_context(tc.tile_pool(name="bcp", bufs=2))
        tmp = es.enter_context(tc.tile_pool(name="tmp", bufs=2))
        lnt = es.enter_context(tc.tile_pool(name="lnt", bufs=1))
        actp = es.enter_context(tc.tile_pool(name="actp", bufs=1))

        # ---- constants ----
        ident_f = const.tile([P, P], F32)
        make_identity(nc, ident_f[:])
        ones_f = const.tile([P, P], F32)
        nc.vector.memset(ones_f[:], 1.0)
        zeros_f = const.tile([P, P], F32)
        nc.vector.memset(zeros_f[:], 0.0)
        ones128 = const.tile([P, 1], F32R)         # stats lhsT (K=128, M=1)
        nc.vector.tensor_copy(ones128[:], ones_f[:, 0:1])
        eps_t = const.tile([1, 1], F32)
        nc.vector.memset(eps_t[:], EPS)
        ws_t = const.tile([P, NC_], F32)           # x32 tile for Pool converts
        nc.vector.memset(ws_t[:], WS)
        ident_r = const.tile([P, P], F32R)
        nc.vector.tensor_copy(ident_r[:], ident_f[:])

        def col_const(name, n):
            t = const.tile([P, n], F32, tag=f"{name}_c")
            nc.sync.dma_start(t[:], vec_in[name].rearrange("(o p) -> p o", p=P))
            return t

        g1c, b1c = col_const("ln1_g", FT), col_const("ln1_b", FT)
        g2c, b2c = col_const("ln2_g", FT), col_const("ln2_b", FT)
        g3c, b3c = col_const("ln3_g", FT), col_const("ln3_b", FT)
        bo1c, bo2c = col_const("bo1", FT), col_const("bo2", FT)
        bfc = col_const("bf", FT)
        bgc = col_const("bg", 2 * NI)   # [:, 0:16]=u biases, [:, 16:32]=g biases
        # u-side bias prescaled by WS (descale happens at the wf epilogue)
        bguc = const.tile([P, NI], F32, tag="bgu32")
        nc.vector.tensor_scalar_mul(bguc[:], bgc[:, 0:NI], WS)
        g1ws = const.tile([P, FT], F32, tag="g1ws")
        nc.vector.tensor_scalar_mul(g1ws[:], g1c[:], WS)
        g2ws = const.tile([P, FT], F32, tag="g2ws")
        nc.vector.tensor_scalar_mul(g2ws[:], g2c[:], WS)

        # ---- load x, PE-transpose into xT [128, FT, S] (f32r) ----
        xT = resid.tile([P, FT, S], F32R, tag="x")
        encT = const.tile([P, KE, EPAD], F8, tag="encT")
        with tc.tile_pool(name="ps_in", bufs=2, space="PSUM") as ps_in, \
             tc.tile_pool(name="encstg", bufs=1) as encstg:
            for st in range(ST):
                xr = stage.tile([P, D], F32, tag="x_raw")
                nc.sync.dma_start(xr[:], x[P * st:P * (st + 1), :])
                pt = ps_in.tile([P, NC_], F32, tag="t")
                for ft in range(FT):
                    nc.tensor.transpose(pt[:, P * ft:P * (ft + 1)],
                                        xr[:, P * ft:P * (ft + 1)], ident_f[:])
                nc.scalar.copy(
                    xT[:, :, P * st:P * (st + 1)],
                    pt[:].rearrange("p (f q) -> p f q", f=FT))

            # ---- enc: PE transposes into encT fp8 (padded pitch) ----
            enc_raw = encstg.tile([SK2, DE], F32, tag="enc_raw")
            nc.sync.dma_start(enc_raw[:], enc[:, :])
            for ke in range(KE):
                pt = ps_in.tile([P, NC_], F32, tag="t")
                nc.tensor.transpose(pt[:, 0:SK2],
                                    enc_raw[:, P * ke:P * (ke + 1)],
                                    ident_f[0:SK2, 0:SK2])
                nc.vector.tensor_copy(encT[:, ke, SK2:EPAD],
                                      zeros_f[:, 0:EPAD - SK2])
                nc.vector.tensor_copy(encT[:, ke, 0:SK2], pt[:, 0:SK2])

        def load_w8(pool, name, kouter, tag, eng, gcol=None, scale_col=None):
            """Stream a [K, N<=512] HBM weight into [128, kouter, N] fp8 (xWS),
            optionally folding a per-input-feature LN gain g into the rows
            (scale_col = g*WS columns, applied via the ACT scale pointer)."""
            dram = w_in[name]
            nout = dram.shape[1]
            wr = pool.tile([P, kouter, nout], F8, tag=tag)
            dram_r = dram.rearrange("(ko ki) n -> ki ko n", ki=P)
            half = (kouter + 1) // 2 if kouter > 4 else kouter
            for k0 in range(0, kouter, half):
                k1 = min(k0 + half, kouter)
                stg = stage.tile([P, half, nout], F32, tag="wst")
                nc.sync.dma_start(stg[:, 0:k1 - k0, :], dram_r[:, k0:k1, :])
                if scale_col is not None:
                    for ko in range(k0, k1):
                        nc.scalar.activation(
                            wr[:, ko, :], stg[:, ko - k0, :], AF.Copy,
                            scale=scale_col[:, ko:ko + 1])
                elif gcol is not None:
                    for ko in range(k0, k1):
                        nc.vector.tensor_scalar(
                            wr[:, ko, :], stg[:, ko - k0, :],
                            gcol[:, ko:ko + 1], WS, op0=ALU.mult, op1=ALU.mult)
                elif eng is nc.scalar:
                    eng.mul(wr[:, k0:k1, :], stg[:, 0:k1 - k0, :], WS)
                else:
                    eng.tensor_scalar_mul(wr[:, k0:k1, :], stg[:, 0:k1 - k0, :], WS)
            return wr

        def ln_chunk(src, bcol, hT, lps, c, st_tag="st", sub_pool=False):
            """Emit LayerNorm chunk c: src f32r -> hT fp8 (g folded into the
            consuming weights, +b applied here). Stats share one psum bank."""
            cs = slice(NC_ * c, NC_ * (c + 1))
            st_ps = lps.tile([1, NC_], F32, tag=st_tag)
            for ft in range(FT):
                nc.tensor.matmul(st_ps[:], ones128[:], src[:, ft, cs],
                                 start=(ft == 0), stop=(ft == FT - 1))
            mu = rowp.tile([1, NC_], F32, tag="mu")
            nc.vector.tensor_scalar_mul(mu[:], st_ps[:], 1.0 / D)
            sq_ps = lps.tile([1, NC_], F32, tag=st_tag, name="sqps")
            for ft in range(FT):
                xsq = tmp.tile([P, NC_], F32R, tag="xsq")
                nc.scalar.activation(xsq[:], src[:, ft, cs], AF.Square)
                nc.tensor.matmul(sq_ps[:], ones128[:], xsq[:],
                                 start=(ft == 0), stop=(ft == FT - 1))
            musq = rowp.tile([1, NC_], F32, tag="musq")
            nc.vector.tensor_mul(musq[:], mu[:], mu[:])
            var = rowp.tile([1, NC_], F32, tag="var")
            nc.vector.scalar_tensor_tensor(
                var[:], sq_ps[:], 1.0 / D, musq[:],
                op0=ALU.mult, op1=ALU.subtract)
            sd = rowp.tile([1, NC_], F32, tag="sd")
            nc.scalar.activation(sd[:], var[:], AF.Sqrt, bias=eps_t[:])
            rstd = rowp.tile([1, NC_], F32, tag="rstd")
            nc.vector.reciprocal(rstd[:], sd[:])
            mu_b = bcp.tile([P, NC_], F32, tag="mub")
            nc.gpsimd.partition_broadcast(mu_b[:], mu[:])
            rstd_b = bcp.tile([P, NC_], F32, tag="rstdb")
            nc.gpsimd.partition_broadcast(rstd_b[:], rstd[:])
            t = lnt.tile([P, FT, NC_], F32R, tag="lt")
            sub_eng = nc.gpsimd if sub_pool else nc.vector
            sub_eng.tensor_tensor(
                t[:], src[:, :, cs],
                mu_b[:, None, :].broadcast_to([P, FT, NC_]), ALU.subtract)
            nc.vector.tensor_tensor(
                t[:], t[:],
                rstd_b[:, None, :].broadcast_to([P, FT, NC_]), ALU.mult)
            nc.vector.tensor_tensor(
                hT[:, :, cs], t[:],
                bcol[:, :, None].broadcast_to([P, FT, NC_]), ALU.add)

        def layer_norm(src, bcol, pool, tag):
            hT = pool.tile([P, FT, S], F8, tag=tag, name=f"h_{tag}")
            with tc.tile_pool(name=f"ps_{tag}", bufs=2, space="PSUM") as lps:
                for c in range(CH):
                    ln_chunk(src, bcol, hT, lps, c)
            return hT

        def project_dr(w_r, src, pool, tag, copy_eng="dve"):
            """yT = (W*WS).T @ src via fp8 DoubleRow, descaled 1/WS on copy.
            src [128, FT, S] fp8; w_r [128, FT, D] fp8. -> [128, FT, S] fp8."""
            yT = pool.tile([P, FT, S], F8, tag=tag)
            with tc.tile_pool(name=f"ps_{tag}", bufs=2, space="PSUM") as pps:
                for c in range(CH):
                    cs = slice(NC_ * c, NC_ * (c + 1))
                    for mo in range(FT):
                        pt = pps.tile([P, NC_], F32, tag="t")
                        for j in range(FT // 2):
                            nc.tensor.matmul(
                                pt[:], w_r[:, 2 * j:2 * j + 2, P * mo:P * (mo + 1)],
                                src[:, 2 * j:2 * j + 2, cs],
                                start=(j == 0), stop=(j == FT // 2 - 1),
                                perf_mode=DR)
                        if copy_eng == "act":
                            nc.scalar.mul(yT[:, mo, cs], pt[:], WSI)
                        else:
                            nc.vector.tensor_scalar_mul(yT[:, mo, cs], pt[:], WSI)
            return yT

        def attention_into(qT, kT, v_sb, n_sk, aT, aT_tag, hook=None,
                           chunk_cb=None, sc_bufs=2, pv_bufs=1):
            """qT/kT: [128, FT, S*] fp8 transposed; v_sb: [part, sk_tiles, NH,
            DH+2] fp8 with ones in col DH. Fills aT [128, FT, S] fp8.
            hook(slot) emits interleaved filler work; chunk_cb(c) emits the
            per-chunk consumer (out-projection) as soon as chunk c is done."""
            sk_tiles = (n_sk + P - 1) // P
            with tc.tile_pool(name=f"ps_sc_{aT_tag}", bufs=sc_bufs,
                              space="PSUM") as psc, \
                 tc.tile_pool(name=f"ps_pv_{aT_tag}", bufs=pv_bufs,
                              space="PSUM") as ppv, \
                 tc.tile_pool(name=f"ex_{aT_tag}", bufs=2) as exp_pool:
                for c in range(CH):
                    for hf in range(FT):
                        cs = slice(NC_ * c, NC_ * (c + 1))
                        ex3 = exp_pool.tile([P, sk_tiles, 2, NC_], F8, tag="ex")
                        for sk in range(sk_tiles):
                            rows = min(P, n_sk - P * sk)
                            sp = psc.tile([P, 2, NC_], F32, tag="sp")
                            for par in range(2):
                                hp = slice(DH * par, DH * par + DH)
                                nc.tensor.matmul(
                                    sp[:rows, par, :],
                                    kT[hp, hf, P * sk:P * sk + rows],
                                    qT[hp, hf, cs], start=True, stop=True)
                            nc.scalar.activation(ex3[:rows, sk, :, :],
                                                 sp[:rows, :, :], AF.Exp,
                                                 scale=SCALE)
                        pv = ppv.tile([DH + 2, 2, NC_], F32, tag="pv")
                        for par in range(2):
                            h = 2 * hf + par
                            if sk_tiles % 2 == 0:
                                for j in range(sk_tiles // 2):
                                    nc.tensor.matmul(
                                        pv[:, par, :],
                                        v_sb[:, 2 * j:2 * j + 2, h, :],
                                        ex3[:, 2 * j:2 * j + 2, par, :],
                                        start=(j == 0),
                                        stop=(j == sk_tiles // 2 - 1),
                                        perf_mode=DR)
                            else:
                                for sk in range(sk_tiles):
                                    rows = min(P, n_sk - P * sk)
                                    nc.tensor.matmul(
                                        pv[:, par, :], v_sb[:rows, sk, h, :],
                                        ex3[:rows, sk, par, :],
                                        start=(sk == 0), stop=(sk == sk_tiles - 1))
                        rc = rowp.tile([1, 2, NC_], F32, tag="rc")
                        nc.vector.reciprocal(rc[:], pv[DH:DH + 1, :, :])
                        for par in range(2):
                            hp = slice(DH * par, DH * par + DH)
                            den_b = bcp.tile([DH, NC_], F32, tag="denb")
                            nc.gpsimd.partition_broadcast(den_b[:], rc[0:1, par, :])
                            nc.vector.tensor_tensor(aT[hp, hf, cs],
                                                    pv[0:DH, par, :],
                                                    den_b[:], ALU.mult)
                        if hook is not None:
                            hook(c * FT + hf)
                    if chunk_cb is not None:
                        chunk_cb(c)

        def out_proj_chunk(ops, w_r, bias_c, aT, src, dst, c):
            """dst[:, :, chunk c] = src + (W*WS).T @ aT / WS + bias."""
            cs = slice(NC_ * c, NC_ * (c + 1))
            for mo in range(FT):
                pt = ops.tile([P, NC_], F32, tag="t")
                for j in range(FT // 2):
                    nc.tensor.matmul(
                        pt[:], w_r[:, 2 * j:2 * j + 2, P * mo:P * (mo + 1)],
                        aT[:, 2 * j:2 * j + 2, cs],
                        start=(j == 0), stop=(j == FT // 2 - 1),
                        perf_mode=DR)
                t1 = tmp.tile([P, NC_], F32R, tag="epi")
                nc.vector.tensor_scalar(
                    t1[:], pt[:], WSI, bias_c[:, mo:mo + 1],
                    op0=ALU.mult, op1=ALU.add)
                nc.vector.tensor_add(dst[:, mo, cs], t1[:], src[:, mo, cs])

        # ================= self-attention =================
        wpool = es.enter_context(tc.tile_pool(name="wpool", bufs=1))
        with tc.tile_pool(name="a1", bufs=1) as a1:
            h1 = layer_norm(xT, b1c, a1, "h1")
            wq1r = load_w8(wpool, "wq1", FT, "wq1r", nc.scalar, scale_col=g1ws)
            wk1r = load_w8(wpool, "wk1", FT, "wk1r", nc.scalar, scale_col=g1ws)
            wv1r = load_w8(wpool, "wv1", FT, "wv1r", nc.scalar, scale_col=g1ws)
            # attn2 enc-side weights: convert on ACT before the exp phase starts
            wk2r = load_w8(wpool, "wk2", KE, "wk2r", nc.scalar)
            wv2r = load_w8(wpool, "wv2", KE, "wv2r", nc.scalar)
            qT1 = project_dr(wq1r, h1, a1, "qT1")
            kT1 = project_dr(wk1r, h1, a1, "kT1")
            v1 = a1.tile([P, ST, NH, DH + 2], F8, tag="v1")
            nc.vector.tensor_copy(
                v1[:, :, :, DH:DH + 1],
                ones_f[:, 0:ST * NH].rearrange("p (a b c) -> p a b c",
                                               a=ST, b=NH))
            nc.vector.tensor_copy(
                v1[:, :, :, DH + 1:DH + 2],
                zeros_f[:, 0:ST * NH].rearrange("p (a b c) -> p a b c",
                                                a=ST, b=NH))
            with tc.tile_pool(name="ps_v1", bufs=2, space="PSUM") as vps:
                for st in range(ST):
                    pt = vps.tile([P, NC_], F32, tag="t")
                    for j in range(FT // 2):
                        nc.tensor.matmul(
                            pt[:], h1[:, 2 * j:2 * j + 2, P * st:P * (st + 1)],
                            wv1r[:, 2 * j:2 * j + 2, :],
                            start=(j == 0), stop=(j == FT // 2 - 1),
                            perf_mode=DR)
                    nc.vector.tensor_scalar_mul(
                        v1[:, st, :, 0:DH],
                        pt[:].rearrange("p (h d) -> p h d", h=NH), WSI)

            wo1r = load_w8(wpool, "wo1", FT, "wo1r", nc.scalar)
            wq2r = load_w8(wpool, "wq2", FT, "wq2r", nc.scalar, scale_col=g2ws)
            wo2r = load_w8(wpool, "wo2", FT, "wo2r", nc.vector)

            # kT2/v2 depend only on enc: compute during attn1
            kT2 = wpool.tile([P, FT, EPAD], F8, tag="kT2")
            v2 = wpool.tile([SK2, 1, NH, DH + 2], F8, tag="v2")
            nc.vector.tensor_copy(
                v2[:, :, :, DH:DH + 1],
                ones_f[0:SK2, 0:NH].rearrange("p (a b c) -> p a b c", a=1, b=NH))
            nc.vector.tensor_copy(
                v2[:, :, :, DH + 1:DH + 2],
                zeros_f[0:SK2, 0:NH].rearrange("p (a b c) -> p a b c", a=1, b=NH))
            with tc.tile_pool(name="ps_kv2", bufs=2, space="PSUM") as kvps:
                for mo in range(FT):
                    pt = kvps.tile([P, NC_], F32, tag="t")
                    for j in range(KE // 2):
                        nc.tensor.matmul(
                            pt[:, 0:EPAD],
                            wk2r[:, 2 * j:2 * j + 2, P * mo:P * (mo + 1)],
                            encT[:, 2 * j:2 * j + 2, :],
                            start=(j == 0), stop=(j == KE // 2 - 1),
                            perf_mode=DR)
                    nc.scalar.mul(kT2[:, mo, :], pt[:, 0:EPAD], WSI)
                pt = kvps.tile([P, NC_], F32, tag="t")
                for j in range(KE // 2):
                    nc.tensor.matmul(pt[:, :], encT[:, 2 * j:2 * j + 2, :],
                                     wv2r[:, 2 * j:2 * j + 2, :],
                                     start=(j == 0), stop=(j == KE // 2 - 1),
                                     perf_mode=DR)
                nc.scalar.mul(
                    v2[:, 0, :, 0:DH],
                    pt[0:SK2, :].rearrange("p (h d) -> p h d", h=NH), WSI)

            # wg streamed + converted on Pool while attn1's ACT runs exp
            wgur = wpool.tile([P, NI, FT, P], F8, tag="wgur")
            wggr = wpool.tile([P, NI, FT, P], F8, tag="wggr")
            wg_r = w_in["wg"].rearrange("(ko ki) n -> ki ko n", ki=P)
            ws_g3 = const.tile([P, FT, P], F32, tag="ws_g3")
            for ft in range(FT):
                nc.vector.tensor_scalar(
                    ws_g3[:, ft, :], ones_f[:, 0:P], g3c[:, ft:ft + 1], WS,
                    op0=ALU.mult, op1=ALU.mult)
            ws_v = ws_g3[:]

            wg_sched = [[0], [1], [2], [3], [4, 5, 6], [7, 8, 9],
                        [10, 11, 12], [13, 14, 15]]

            def wg_hook(slot):
                for i in wg_sched[slot]:
                    wgu = wgstg.tile([P, FT, P], F32, tag="wgst")
                    nc.sync.dma_start(wgu[:], wg_r[:, :, P * i:P * (i + 1)])
                    nc.gpsimd.tensor_tensor(wgur[:, i, :, :], wgu[:], ws_v,
                                            ALU.mult)
                    wgg = wgstg.tile([P, FT, P], F32, tag="wgst")
                    nc.sync.dma_start(
                        wgg[:], wg_r[:, :, FF + P * i:FF + P * (i + 1)])
                    nc.gpsimd.tensor_tensor(wggr[:, i, :, :], wgg[:], ws_v,
                                            ALU.mult)

            xT1 = resid.tile([P, FT, S], F32R, tag="x")
            h2 = actp.tile([P, FT, S], F8, tag="h2")
            aT1_box = []

            def o1_cb(c):
                out_proj_chunk(ops1, wo1r, bo1c, aT1_box[0], xT, xT1, c)
                ln_chunk(xT1, b2c, h2, ops1, c, sub_pool=True)

            with tc.tile_pool(name="ps_o1", bufs=1, space="PSUM") as ops1:
                aT1_box.append(a1.tile([P, FT, S], F8, tag="aT1", name="aT1"))
                attention_into(qT1, kT1, v1, S, aT1_box[0], "aT1", hook=wg_hook,
                               chunk_cb=o1_cb)

        # ================= cross-attention =================
        with tc.tile_pool(name="a2", bufs=1) as a2:
            qT2 = project_dr(wq2r, h2, a2, "qT2", copy_eng="act")

            # wf streamed + converted on Pool during attn2
            wfr = wpool.tile([P, NI, D], F8, tag="wfr")
            wf_r = w_in["wf"].rearrange("(ko ki) n -> ki ko n", ki=P)

            def wf_hook(slot):
                if slot >= FT:
                    return
                kq = slot
                wf_stg = wfstg.tile([P, FT, D], F32, tag="wfst")
                nc.sync.dma_start(wf_stg[:], wf_r[:, FT * kq:FT * (kq + 1), :])
                for ko in range(FT):
                    nc.gpsimd.tensor_tensor(
                        wfr[:, FT * kq + ko, :], wf_stg[:, ko, :], ws_t[:],
                        ALU.mult)

            xT2 = resid.tile([P, FT, S], F32R, tag="x")
            h3 = actp.tile([P, FT, S], F8, tag="h3")
            aT2_box = []

            def o2_cb(c):
                out_proj_chunk(ops2, wo2r, bo2c, aT2_box[0], xT1, xT2, c)
                ln_chunk(xT2, b3c, h3, ops2, c, sub_pool=True)

            with tc.tile_pool(name="ps_o2", bufs=1, space="PSUM") as ops2:
                aT2_box.append(a2.tile([P, FT, S], F8, tag="aT2", name="aT2"))
                attention_into(qT2, kT2, v2, SK2, aT2_box[0], "aT2", hook=wf_hook,
                               chunk_cb=o2_cb, sc_bufs=1, pv_bufs=2)

        # ================= GEGLU feed-forward =================
        with tc.tile_pool(name="ffp", bufs=1) as ffp, \
             tc.tile_pool(name="fft_p", bufs=4) as fft_p, \
             tc.tile_pool(name="gel_p", bufs=4) as gel_p:
            xT3 = resid.tile([P, FT, S], F32R, tag="x")
            with tc.tile_pool(name="ps_ff", bufs=4, space="PSUM") as ffps, \
                 tc.tile_pool(name="ps_wf", bufs=4, space="PSUM") as wfps:
                for c in range(CH):
                    cs = slice(NC_ * c, NC_ * (c + 1))
                    wf_ps = [wfps.tile([P, NC_], F32, tag="a",
                                       name=f"wf_ps{c}_{m}") for m in range(FT)]
                    for i in range(NI):
                        pu = ffps.tile([P, NC_], F32, tag="t")
                        for j in range(FT // 2):
                            nc.tensor.matmul(pu[:],
                                             wgur[:, i, 2 * j:2 * j + 2, :],
                                             h3[:, 2 * j:2 * j + 2, cs],
                                             start=(j == 0), stop=(j == FT // 2 - 1),
                                             perf_mode=DR)
                        pg = ffps.tile([P, NC_], F32, tag="t")
                        for j in range(FT // 2):
                            nc.tensor.matmul(pg[:],
                                             wggr[:, i, 2 * j:2 * j + 2, :],
                                             h3[:, 2 * j:2 * j + 2, cs],
                                             start=(j == 0), stop=(j == FT // 2 - 1),
                                             perf_mode=DR)
                        gel = gel_p.tile([P, NC_], BF16, tag="gel")
                        nc.scalar.activation(gel[:], pg[:], AF.Gelu,
                                             bias=bgc[:, NI + i:NI + i + 1],
                                             scale=WSI)
                        if i % 2 == 0:
                            fpair = fft_p.tile([P, 2, NC_], F8, tag="fpair")
                        # fft = (pu + 32*bg_u) * gel = 32 * (u+bu)*gelu(g+bgg)
                        nc.vector.scalar_tensor_tensor(
                            fpair[:, i % 2, :], pu[:], bguc[:, i:i + 1], gel[:],
                            op0=ALU.add, op1=ALU.mult)
                        if i % 2 == 1:
                            for m in range(FT):
                                nc.tensor.matmul(
                                    wf_ps[m][:],
                                    wfr[:, i - 1:i + 1, P * m:P * (m + 1)],
                                    fpair[:], start=(i == 1), stop=(i == NI - 1),
                                    perf_mode=DR, skip_group_check=True)
                    for m in range(FT):
                        t1 = tmp.tile([P, NC_], F32R, tag="epi")
                        nc.vector.tensor_scalar(
                            t1[:], wf_ps[m][:], WSI * WSI, bfc[:, m:m + 1],
                            op0=ALU.mult, op1=ALU.add)
                        nc.gpsimd.tensor_add(xT3[:, m, cs], t1[:], xT2[:, m, cs])
                    # transpose back & store this chunk immediately
                    for st in range(4 * c, 4 * c + 4):
                        ot = stage.tile([P, D], F32, tag="x_raw")
                        pt = ffps.tile([P, NC_], F32R, tag="t")
                        for ft in range(FT):
                            nc.tensor.transpose(pt[:, P * ft:P * (ft + 1)],
                                                xT3[:, ft, P * st:P * (st + 1)],
                                                ident_r[:])
                        nc.vector.tensor_copy(ot[:], pt[:])
                        nc.sync.dma_start(out[P * st:P * (st + 1), :], ot[:])

    return nc


_CACHED = {}


def _get_nc():
    if "nc" not in _CACHED:
        nc = bacc.Bacc("TRN2", target_bir_lowering=False, debug=False, num_devices=B)
        build(nc)
        nc.finalize()
        _CACHED["nc"] = nc
    return _CACHED["nc"]


def kernel(**inputs) -> np.ndarray:
    nc = _get_nc()
    x = np.ascontiguousarray(np.asarray(inputs["x"]), dtype=np.float32)
    enc = np.ascontiguousarray(np.asarray(inputs["enc"]), dtype=np.float32)
    shared = {k: np.ascontiguousarray(np.asarray(v), dtype=np.float32)
              for k, v in inputs.items() if k not in ("x", "enc")}
    in_maps = [dict(shared, x=x[i], enc=enc[i]) for i in range(B)]
    res = run_bass_kernel_spmd(nc, in_maps, core_ids=list(range(B)))
    outs = []
    for i in range(B):
        o = np.asarray(res.results[i]["out"])
        outs.append(o.view(np.float32) if o.dtype != np.float32 else o)
    return np.stack(outs, axis=0)


if __name__ == "__main__":
    print("module import ok")
